# revision 14
# baseline (speedup 1.0000x reference)
"""CrossAttnBlock Trainium2 kernel (8 NeuronCores, SPMD).

Sharding: query-sequence parallel. 16384 query rows (B*Lq) are split 8 ways;
cores 0-3 take batch 0, cores 4-7 take batch 1 (2048 rows each). The small
ragged KV (N=512) projection is contraction-sharded 4 ways inside each batch
group and AllReduce'd; each core only materializes its own batch's KV segment
plus the probe columns.

Layout plan (per core, T=2048 query tokens):
  qT   = host-pretransposed q shard               [E, T]    (f32r, direct DMA)
  qpT  = W_qT.T @ qT + b_q (per-partition add)    [E, T]    psum->sbuf
  kT   = kv-proj, k half, transposed layout       [E, S+P]  (AllReduce'd)
         probe cols use W_k (batch 0) / W_v + v_bias (batch 1) -- the
         reference's einsum 'bhqd,bhpd' binds batch to the k/v axis of kv.
  v    = kv-proj, v half, natural layout          [S, E]    (AllReduce'd)
  LT   = kT_h.T @ qpT_h per head                  [S, T]    psum (transposed)
  PT   = exp(scale*LT + mask_bias)                [S, T]    f32r sbuf
  den  = ones.T @ PT per head -> one [H, T] psum; single reciprocal
  oT   = v_h.T @ PT (unnormalized)                [D, T]    psum
  out  = oT * gpsimd-broadcast(1/den_h)           [E, T]    sbuf
  y    = outT.T @ W_projT (+b_eff via ones row)   [T, E]    psum->sbuf->DRAM
  probes: natural-layout logits [T, 4, P] per head, one exp per head-chunk,
          batched free-dim max/sum reduces -> imp = mean_h(max/den).

v_bias enters the attention output only through b_eff = W_proj@v_bias + b_proj
(softmax rows sum to 1), applied via an appended ones row on the proj
stationary. All PE-facing tensors are float32r (fp32 bits, reduced-precision
full-rate streaming); PSUM accumulation stays fp32.
"""

import functools
import sys

import numpy as np

try:
    import concourse.bass as bass  # noqa: F401
except ImportError:
    sys.path.insert(0, "/opt/trn_rl_repo")

import concourse.bass as bass
import concourse.tile as tile
from concourse import bacc, mybir
from concourse.bass_utils import run_bass_kernel_spmd

F32 = mybir.dt.float32
F32R = mybir.dt.float32r

NCORES = 8
P128 = 128


@functools.lru_cache(maxsize=2)
def _build(T, SP, P, KC, E, H, D, use_f32r, use_cc=True):
    """Build + compile the per-core Bass module. All shapes are compile-time.

    T  : query tokens per core (2048)
    SP : padded KV segment length (mult of 128)
    P  : number of probe ids
    KC : contraction rows of the kv projection handled per core (KVD/4)
    """
    assert T % 512 == 0 and SP % P128 == 0 and E % P128 == 0
    HD1 = D + 1         # per-head v width incl. ones column
    VA = H * HD1        # 780
    NT = SP // P128     # kv n-tiles (2)
    ET = E // P128      # feature tiles (6)
    TC = T // 512       # 512-token chunks (4)
    KCT = KC // P128    # kv-proj K tiles per core (8) + 1 single-row
    scale = 1.0 / np.sqrt(D)
    RD = F32R if use_f32r else F32

    nc = bacc.Bacc("TRN2", target_bir_lowering=False, debug=False,
                   num_devices=NCORES)

    qT_d = nc.dram_tensor("qT_s", [E, T], RD, kind="ExternalInput").ap()
    kvcT_d = nc.dram_tensor("kvcT", [KC + 1, SP + P], RD, kind="ExternalInput").ap()
    wkT_d = nc.dram_tensor("wkT", [KC + 1, E], RD, kind="ExternalInput").ap()
    wvT_d = nc.dram_tensor("wvT", [KC + 1, VA], RD, kind="ExternalInput").ap()
    wpkT_d = nc.dram_tensor("wpkT", [KC + 1, E], RD, kind="ExternalInput").ap()
    wqT_d = nc.dram_tensor("wqT", [E, E], RD, kind="ExternalInput").ap()
    wpT_d = nc.dram_tensor("wpT", [E + 1, E], RD, kind="ExternalInput").ap()
    bq_d = nc.dram_tensor("bq", [E // P128, P128], F32, kind="ExternalInput").ap()
    mask_d = nc.dram_tensor("maskns", [SP // P128, P128], F32,
                            kind="ExternalInput").ap()
    ones_d = nc.dram_tensor("ones", [1, 512], RD, kind="ExternalInput").ap()
    y_d = nc.dram_tensor("y_s", [T, E], F32, kind="ExternalOutput").ap()
    imp_d = nc.dram_tensor("imp_s", [T // P128, P128], F32, kind="ExternalOutput").ap()

    groups = [[0, 1, 2, 3], [4, 5, 6, 7]]

    with tile.TileContext(nc) as tc:
        from contextlib import ExitStack
        with ExitStack() as ctx:
            const_p = ctx.enter_context(tc.tile_pool(name="const", bufs=1))
            pers_p = ctx.enter_context(tc.tile_pool(name="pers", bufs=1))
            dram_p = ctx.enter_context(tc.tile_pool(name="dram", bufs=1, space="DRAM"))

            ones_row = const_p.tile([1, 512], RD)
            nc.sync.dma_start(ones_row[:], ones_d[:])
            ones_col = const_p.tile([P128, 1], RD)
            nc.sync.dma_start(ones_col[:], ones_d[0, :P128])
            mask_sb = const_p.tile([P128, NT], F32)
            for j in range(NT):
                nc.sync.dma_start(mask_sb[:, j:j + 1], mask_d[j, :])
            bq_sb = const_p.tile([P128, ET], F32)
            for j in range(ET):
                nc.sync.dma_start(bq_sb[:, j:j + 1], bq_d[j, :])

            # ---- persistent tensors ----
            qT = [pers_p.tile([P128, T], RD, name=f"qT{i}") for i in range(ET)]
            for i in range(ET):
                nc.sync.dma_start(qT[i][:], qT_d[i * P128:(i + 1) * P128, :])
            wq_sb = [pers_p.tile([P128, E], RD, name=f"wq{i}") for i in range(ET)]
            wp_sb = [pers_p.tile([P128, E], RD, name=f"wp{i}") for i in range(ET)]
            wp_last = pers_p.tile([1, E], RD)
            for i in range(ET):
                nc.sync.dma_start(wq_sb[i][:], wqT_d[i * P128:(i + 1) * P128, :])
                nc.sync.dma_start(wp_sb[i][:], wpT_d[i * P128:(i + 1) * P128, :])
            nc.sync.dma_start(wp_last[:], wpT_d[E:E + 1, :])

            kT_sb = [pers_p.tile([P128, SP + P], RD, name=f"kT{i}") for i in range(ET)]
            v_sb = [pers_p.tile([P128, VA], RD, name=f"vsb{i}") for i in range(NT)]
            imp_sb = pers_p.tile([P128, T // P128], F32)

            # ================= Phase A: kv projection (sharded) ===========
            with ExitStack() as actx:
                akvc_p = actx.enter_context(tc.tile_pool(name="akvc", bufs=KCT + 1))
                aw_p = actx.enter_context(tc.tile_pool(name="aw", bufs=3))
                aps_p = actx.enter_context(
                    tc.tile_pool(name="aps", bufs=8, space="PSUM"))
                asb_p = actx.enter_context(tc.tile_pool(name="asb", bufs=4))

                kvc = []
                for kc in range(KCT + 1):
                    pr = P128 if kc < KCT else 1
                    t = akvc_p.tile([pr, SP + P], RD, name=f"kvc{kc}", tag="kvc")
                    nc.sync.dma_start(t[:], kvcT_d[kc * P128:kc * P128 + pr, :])
                    kvc.append(t)

                kT_in = dram_p.tile([E, SP + P], F32)
                v_in = dram_p.tile([SP, VA], F32)
                if use_cc:
                    kT_out = dram_p.tile([E, SP + P], F32)
                    v_out = dram_p.tile([SP, VA], F32)
                else:
                    kT_out, v_out = kT_in, v_in

                # A1: kT[o, n] = sum_i wkT[i, o] * kvcT[i, n]  (segment cols)
                kps = [aps_p.tile([P128, SP], F32, name=f"kps{o}", tag="ps")
                       for o in range(ET)]
                for kc in range(KCT + 1):
                    pr = P128 if kc < KCT else 1
                    wk = aw_p.tile([pr, E], RD, name="wk", tag="wk")
                    nc.sync.dma_start(wk[:], wkT_d[kc * P128:kc * P128 + pr, :])
                    for o in range(ET):
                        nc.tensor.matmul(
                            kps[o][:], wk[:, o * P128:(o + 1) * P128],
                            kvc[kc][:, :SP], start=(kc == 0), stop=(kc == KCT))
                # A1b: probe cols use the probe weight (K for batch0, V for batch1)
                pps = [aps_p.tile([P128, P], F32, name=f"pps{o}", tag="ps")
                       for o in range(ET)]
                for kc in range(KCT + 1):
                    pr = P128 if kc < KCT else 1
                    wpk = aw_p.tile([pr, E], RD, name="wpk", tag="wpk")
                    nc.sync.dma_start(wpk[:], wpkT_d[kc * P128:kc * P128 + pr, :])
                    for o in range(ET):
                        nc.tensor.matmul(
                            pps[o][:], wpk[:, o * P128:(o + 1) * P128],
                            kvc[kc][:, SP:SP + P], start=(kc == 0),
                            stop=(kc == KCT))
                for o in range(ET):
                    ksb = asb_p.tile([P128, SP + P], F32, name="ksb", tag="ksb")
                    nc.vector.tensor_copy(ksb[:, :SP], kps[o][:])
                    nc.vector.tensor_copy(ksb[:, SP:], pps[o][:])
                    nc.scalar.dma_start(kT_in[o * P128:(o + 1) * P128, :], ksb[:])

                # A2: v[n, j] = sum_i kvcT[i, n] * wvT[i, j]
                vps = [aps_p.tile([P128, VA // 2], F32, name=f"vps{i}", tag="ps")
                       for i in range(NT * 2)]
                for kc in range(KCT + 1):
                    pr = P128 if kc < KCT else 1
                    wv = aw_p.tile([pr, VA], RD, name="wv", tag="wv")
                    nc.sync.dma_start(wv[:], wvT_d[kc * P128:kc * P128 + pr, :])
                    for n in range(NT):
                        for hf in range(2):
                            nc.tensor.matmul(
                                vps[n * 2 + hf][:],
                                kvc[kc][:, n * P128:(n + 1) * P128],
                                wv[:, hf * (VA // 2):(hf + 1) * (VA // 2)],
                                start=(kc == 0), stop=(kc == KCT))
                for n in range(NT):
                    vsb = asb_p.tile([P128, VA], F32, name="vsb", tag="vsb")
                    for hf in range(2):
                        nc.vector.tensor_copy(
                            vsb[:, hf * (VA // 2):(hf + 1) * (VA // 2)],
                            vps[n * 2 + hf][:])
                    nc.scalar.dma_start(v_in[n * P128:(n + 1) * P128, :], vsb[:])

                if use_cc:
                    nc.gpsimd.collective_compute(
                        "AllReduce", mybir.AluOpType.add, replica_groups=groups,
                        ins=[kT_in[:]], outs=[kT_out[:]])
                    nc.gpsimd.collective_compute(
                        "AllReduce", mybir.AluOpType.add, replica_groups=groups,
                        ins=[v_in[:]], outs=[v_out[:]])

                for o in range(ET):
                    nc.sync.dma_start(
                        kT_sb[o][:], kT_out[o * P128:(o + 1) * P128, :].bitcast(RD))
                for n in range(NT):
                    nc.sync.dma_start(
                        v_sb[n][:], v_out[n * P128:(n + 1) * P128, :].bitcast(RD))

            # ================= Phase B: main pipeline =====================
            qp_p = ctx.enter_context(tc.tile_pool(name="qp", bufs=2))
            pt_p = ctx.enter_context(tc.tile_pool(name="pt", bufs=2))
            ot_p = ctx.enter_context(tc.tile_pool(name="ot", bufs=2))
            sm_p = ctx.enter_context(tc.tile_pool(name="sm", bufs=3))
            pr_p = ctx.enter_context(tc.tile_pool(name="pr", bufs=3))
            ysb_p = ctx.enter_context(tc.tile_pool(name="ysb", bufs=3))
            ps_p = ctx.enter_context(tc.tile_pool(name="ps", bufs=4, space="PSUM"))
            op_p = ctx.enter_context(tc.tile_pool(name="op", bufs=4, space="PSUM"))

            for tcn in range(TC):
                t0 = tcn * 512
                # ---- q projection -> qpT [o, t] (+ b_q per-partition) ----
                qpT = [qp_p.tile([P128, 512], RD, name=f"qpT{i}", tag=f"qpT{i}")
                       for i in range(ET)]
                for o in range(ET):
                    qps = ps_p.tile([P128, 512], F32, name="qps", tag="ps")
                    for ic in range(ET):
                        nc.tensor.matmul(
                            qps[:], wq_sb[ic][:, o * P128:(o + 1) * P128],
                            qT[ic][:, t0:t0 + 512],
                            start=(ic == 0), stop=(ic == ET - 1))
                    nc.vector.tensor_scalar_add(qpT[o][:], qps[:],
                                                bq_sb[:, o:o + 1])

                # ---- attention: logits+exp+denominators, then AV ----
                outT = [ot_p.tile([P128, 512], RD, name=f"outT{i}", tag=f"outT{i}")
                        for i in range(ET)]
                HB = 4  # heads per batch (bounds live psum tiles)
                for hb in range(H // HB):
                    den_all = sm_p.tile([HB, 512], F32, name="den_all",
                                        tag="den_all")
                    ops = []
                    for hh in range(HB):
                        h = hb * HB + hh
                        et, ro = (h * D) // P128, (h * D) % P128
                        qph = qpT[et][ro:ro + D, :]
                        PT = [pt_p.tile([P128, 512], RD, name=f"PT{n}",
                                        tag=f"PT{n}") for n in range(NT)]
                        for n in range(NT):
                            lt = ps_p.tile([P128, 512], F32, name="lt", tag="ps")
                            nc.tensor.matmul(
                                lt[:],
                                kT_sb[et][ro:ro + D, n * P128:(n + 1) * P128],
                                qph, start=True, stop=True)
                            nc.scalar.activation(
                                PT[n][:], lt[:],
                                mybir.ActivationFunctionType.Exp,
                                bias=mask_sb[:, n:n + 1], scale=float(scale))
                        op = op_p.tile([HD1, 512], F32, name="op", tag="op")
                        for n in range(NT):
                            nc.tensor.matmul(
                                op[:], v_sb[n][:, h * HD1:(h + 1) * HD1],
                                PT[n][:], start=(n == 0), stop=(n == NT - 1))
                        nc.vector.tensor_copy(den_all[hh:hh + 1, :],
                                              op[D:D + 1, :])
                        ops.append(op)
                    rden = sm_p.tile([HB, 512], F32, name="rden", tag="rden")
                    nc.vector.reciprocal(rden[:], den_all[:])
                    for hh in range(HB):
                        h = hb * HB + hh
                        et, ro = (h * D) // P128, (h * D) % P128
                        rbc = sm_p.tile([D, 512], F32, name="rbc", tag="rbc")
                        nc.gpsimd.partition_broadcast(rbc[:], rden[hh:hh + 1, :])
                        nc.vector.tensor_mul(outT[et][ro:ro + D, :],
                                             ops[hh][0:D, :], rbc[:])

                # ---- probes: logits [t, 4, P] per head, batched stats ----
                maxPH = pr_p.tile([P128, 4, H], F32, name="maxPH", tag="maxPH")
                denPH = pr_p.tile([P128, 4, H], F32, name="denPH", tag="denPH")
                for h in range(H):
                    et, ro = (h * D) // P128, (h * D) % P128
                    pl = ps_p.tile([P128, 4, P], F32, name="pl", tag="ps")
                    for tt in range(4):
                        nc.tensor.matmul(
                            pl[:, tt, :],
                            qpT[et][ro:ro + D, tt * P128:(tt + 1) * P128],
                            kT_sb[et][ro:ro + D, SP:SP + P],
                            start=True, stop=True)
                    sp = pr_p.tile([P128, 4, P], F32, name="sp", tag="sp")
                    nc.scalar.activation(
                        sp[:], pl[:], mybir.ActivationFunctionType.Exp)
                    nc.vector.tensor_reduce(
                        maxPH[:, :, h], sp[:], axis=mybir.AxisListType.X,
                        op=mybir.AluOpType.max)
                    nc.vector.tensor_reduce(
                        denPH[:, :, h], sp[:], axis=mybir.AxisListType.X,
                        op=mybir.AluOpType.add)
                rcp = pr_p.tile([P128, 4, H], F32, name="rcp", tag="rcp")
                nc.vector.reciprocal(rcp[:], denPH[:])
                iv = pr_p.tile([P128, 4, H], F32, name="iv", tag="iv")
                nc.vector.tensor_mul(iv[:], maxPH[:], rcp[:])
                red = pr_p.tile([P128, 4], F32, name="red", tag="red")
                nc.vector.tensor_reduce(
                    red[:], iv[:], axis=mybir.AxisListType.X,
                    op=mybir.AluOpType.add)
                nc.vector.tensor_scalar_mul(
                    imp_sb[:, tcn * 4:tcn * 4 + 4], red[:], 1.0 / H)

                # ---- output projection ----
                for tt in range(4):
                    ysb = ysb_p.tile([P128, E], F32, name="ysb", tag="ysb")
                    for hf in range(2):
                        yps = ps_p.tile([P128, E // 2], F32, name="yps", tag="ps")
                        for ec in range(ET):
                            nc.tensor.matmul(
                                yps[:],
                                outT[ec][:, tt * P128:(tt + 1) * P128],
                                wp_sb[ec][:, hf * (E // 2):(hf + 1) * (E // 2)],
                                start=(ec == 0), stop=False)
                        nc.tensor.matmul(
                            yps[:], ones_row[:, tt * P128:(tt + 1) * P128],
                            wp_last[:, hf * (E // 2):(hf + 1) * (E // 2)],
                            start=False, stop=True)
                        nc.scalar.copy(ysb[:, hf * (E // 2):(hf + 1) * (E // 2)],
                                       yps[:])
                    nc.scalar.dma_start(
                        y_d[t0 + tt * P128:t0 + (tt + 1) * P128, :], ysb[:])

            for j in range(T // P128):
                nc.scalar.dma_start(imp_d[j, :], imp_sb[:, j:j + 1])

    nc.compile()
    return nc


def kernel(q, kv_compact, W_q, b_q, W_kv, v_bias, W_proj, b_proj,
           cu_seqlens_k, max_seqlen_k, probe_ids):
    q = np.asarray(q, np.float32)
    kv_compact = np.asarray(kv_compact, np.float32)
    W_q = np.asarray(W_q, np.float32)
    b_q = np.asarray(b_q, np.float32)
    W_kv = np.asarray(W_kv, np.float32)
    v_bias = np.asarray(v_bias, np.float32)
    W_proj = np.asarray(W_proj, np.float32)
    b_proj = np.asarray(b_proj, np.float32)
    cu = np.asarray(cu_seqlens_k, np.int64)
    probe_ids = np.asarray(probe_ids, np.int64)

    B, LQ, E = q.shape
    N, KVD = kv_compact.shape
    H = 12
    D = E // H
    assert B == 2 and NCORES % B == 0
    CPB = NCORES // B            # cores per batch
    T = (B * LQ) // NCORES       # tokens per core
    P = len(probe_ids)
    KC = KVD // CPB
    S = [int(cu[b + 1] - cu[b]) for b in range(B)]
    SP = max(P128, -(-max(S) // P128) * P128)

    # ---------- host-side input prep ----------
    kvT = np.ascontiguousarray(kv_compact.T)               # [KVD, N]
    probeT = kvT[:, probe_ids]                             # [KVD, P]
    wkT_full = np.ascontiguousarray(W_kv[:E].T)            # [KVD, E]
    wvT_full = np.ascontiguousarray(W_kv[E:].T)            # [KVD, E]
    wqT = np.ascontiguousarray(W_q.T)
    HD1 = D + 1
    VA = H * HD1
    wvT_aug = np.zeros((KVD + 1, VA), np.float32)
    for h in range(H):
        wvT_aug[:KVD, h * HD1:h * HD1 + D] = wvT_full[:, h * D:(h + 1) * D]
        wvT_aug[KVD, h * HD1 + D] = 1.0
    b_eff = W_proj @ v_bias + b_proj
    wpT = np.concatenate([np.ascontiguousarray(W_proj.T), b_eff[None, :]], 0)

    in_maps = []
    for c in range(NCORES):
        b, g = c // CPB, c % CPB
        lo, hi = int(cu[b]), int(cu[b + 1])
        segT = np.zeros((KVD, SP), np.float32)
        segT[:, :hi - lo] = kvT[:, lo:hi]
        kvcT_aug = np.concatenate([segT, probeT], 1)       # [KVD, SP+P]
        kvcT = np.zeros((KC + 1, SP + P), np.float32)
        kvcT[:KC] = kvcT_aug[g * KC:(g + 1) * KC]
        if g == 0:
            kvcT[KC] = 1.0    # ones row: v denominator + probe v_bias source
        wkT = np.zeros((KC + 1, E), np.float32)
        wkT[:KC] = wkT_full[g * KC:(g + 1) * KC]
        wvT = np.zeros((KC + 1, VA), np.float32)
        wvT[:KC] = wvT_aug[g * KC:(g + 1) * KC]
        if g == 0:
            wvT[KC] = wvT_aug[KVD]
        # probe projection: batch 0 probes against K, batch 1 against V(+bias)
        wpk_full = wkT_full if b == 0 else wvT_full
        wpkT = np.zeros((KC + 1, E), np.float32)
        wpkT[:KC] = wpk_full[g * KC:(g + 1) * KC]
        if g == 0 and b == 1:
            wpkT[KC] = v_bias
        mask = np.zeros((SP // P128, P128), np.float32)
        mask.reshape(-1)[hi - lo:] = -1e30
        in_maps.append({
            "qT_s": np.ascontiguousarray(q[b, g * T:(g + 1) * T].T),
            "kvcT": kvcT, "wkT": wkT, "wvT": wvT, "wpkT": wpkT,
            "wqT": wqT, "wpT": wpT, "maskns": mask,
            "bq": np.ascontiguousarray(b_q.reshape(E // P128, P128)),
            "ones": np.ones((1, 512), np.float32),
        })

    nc = _build(T, SP, P, KC, E, H, D, True)
    res = run_bass_kernel_spmd(nc, in_maps, core_ids=list(range(NCORES)))

    out = np.empty((B, LQ, E), np.float32)
    imp = np.empty((B, LQ), np.float32)
    for c in range(NCORES):
        b, g = c // CPB, c % CPB
        out[b, g * T:(g + 1) * T] = res.results[c]["y_s"]
        imp[b, g * T:(g + 1) * T] = res.results[c]["imp_s"].reshape(-1)
    return out, imp


# revision 17
# speedup vs baseline: 1.1009x; 1.1009x over previous
"""CrossAttnBlock Trainium2 kernel (8 NeuronCores, SPMD).

Sharding: query-sequence parallel. 16384 query rows (B*Lq) are split 8 ways;
cores 0-3 take batch 0, cores 4-7 take batch 1 (2048 rows each). The small
ragged KV (N=512) projection is contraction-sharded 4 ways inside each batch
group and AllReduce'd; each core only materializes its own batch's KV segment
plus the probe columns.

Layout plan (per core, T=2048 query tokens):
  qT   = host-pretransposed q shard               [E, T]    (f32r, direct DMA)
  qpT  = W_qT.T @ qT + b_q (per-partition add)    [E, T]    psum->sbuf
  kT   = kv-proj, k half, transposed layout       [E, S+P]  (AllReduce'd)
         probe cols use W_k (batch 0) / W_v + v_bias (batch 1) -- the
         reference's einsum 'bhqd,bhpd' binds batch to the k/v axis of kv.
  v    = kv-proj, v half, natural layout          [S, E]    (AllReduce'd)
  LT   = kT_h.T @ qpT_h per head                  [S, T]    psum (transposed)
  PT   = exp(scale*LT + mask_bias)                [S, T]    f32r sbuf
  den  = ones.T @ PT per head -> one [H, T] psum; single reciprocal
  oT   = v_h.T @ PT (unnormalized)                [D, T]    psum
  out  = oT * gpsimd-broadcast(1/den_h)           [E, T]    sbuf
  y    = outT.T @ W_projT (+b_eff via ones row)   [T, E]    psum->sbuf->DRAM
  probes: natural-layout logits [T, 4, P] per head, one exp per head-chunk,
          batched free-dim max/sum reduces -> imp = mean_h(max/den).

v_bias enters the attention output only through b_eff = W_proj@v_bias + b_proj
(softmax rows sum to 1), applied via an appended ones row on the proj
stationary. All PE-facing tensors are float32r (fp32 bits, reduced-precision
full-rate streaming); PSUM accumulation stays fp32.
"""

import functools
import sys

import numpy as np

try:
    import concourse.bass as bass  # noqa: F401
except ImportError:
    sys.path.insert(0, "/opt/trn_rl_repo")

import concourse.bass as bass
import concourse.tile as tile
from concourse import bacc, mybir
from concourse.bass_utils import run_bass_kernel_spmd

F32 = mybir.dt.float32
F32R = mybir.dt.float32r

NCORES = 8
P128 = 128


@functools.lru_cache(maxsize=4)
def _build(T, SP, P, KC, E, H, D, use_f32r, use_cc=True, bufs_cfg=()):
    """Build + compile the per-core Bass module. All shapes are compile-time.

    T  : query tokens per core (2048)
    SP : padded KV segment length (mult of 128)
    P  : number of probe ids
    KC : contraction rows of the kv projection handled per core (KVD/4)
    """
    assert T % 512 == 0 and SP % P128 == 0 and E % P128 == 0
    HD1 = D + 1         # per-head v width incl. ones column
    VA = H * HD1        # 780
    NT = SP // P128     # kv n-tiles (2)
    ET = E // P128      # feature tiles (6)
    TC = T // 512       # 512-token chunks (4)
    KCT = KC // P128    # kv-proj K tiles per core (8) + 1 single-row
    scale = 1.0 / np.sqrt(D)
    RD = F32R if use_f32r else F32
    cfg = dict(bufs_cfg)
    B_qp = cfg.get("qp", 5); B_pt = cfg.get("pt", 2); B_ot = cfg.get("ot", 2)
    B_ps = cfg.get("ps", 4); B_op = cfg.get("op", 4); B_ysb = cfg.get("ysb", 3)
    HB = cfg.get("HB", 4)

    nc = bacc.Bacc("TRN2", target_bir_lowering=False, debug=False,
                   num_devices=NCORES)

    qT_d = nc.dram_tensor("qT_s", [E, T], RD, kind="ExternalInput").ap()
    kvcT_d = nc.dram_tensor("kvcT", [KC + 1, SP + P], RD, kind="ExternalInput").ap()
    wkT_d = nc.dram_tensor("wkT", [KC + 1, E], RD, kind="ExternalInput").ap()
    wvT_d = nc.dram_tensor("wvT", [KC + 1, VA], RD, kind="ExternalInput").ap()
    wpkT_d = nc.dram_tensor("wpkT", [KC + 1, E], RD, kind="ExternalInput").ap()
    wqT_d = nc.dram_tensor("wqT", [E, E], RD, kind="ExternalInput").ap()
    wpT_d = nc.dram_tensor("wpT", [E + 1, E], RD, kind="ExternalInput").ap()
    bq_d = nc.dram_tensor("bq", [E // P128, P128], F32, kind="ExternalInput").ap()
    mask_d = nc.dram_tensor("maskns", [SP // P128, P128], F32,
                            kind="ExternalInput").ap()
    ones_d = nc.dram_tensor("ones", [1, 512], RD, kind="ExternalInput").ap()
    y_d = nc.dram_tensor("y_s", [T, E], F32, kind="ExternalOutput").ap()
    imp_d = nc.dram_tensor("imp_s", [T // P128, P128], F32, kind="ExternalOutput").ap()

    groups = [[0, 1, 2, 3], [4, 5, 6, 7]]

    with tile.TileContext(nc) as tc:
        from contextlib import ExitStack
        with ExitStack() as ctx:
            const_p = ctx.enter_context(tc.tile_pool(name="const", bufs=1))
            pers_p = ctx.enter_context(tc.tile_pool(name="pers", bufs=1))
            dram_p = ctx.enter_context(tc.tile_pool(name="dram", bufs=1, space="DRAM"))

            ones_row = const_p.tile([1, 512], RD)
            nc.sync.dma_start(ones_row[:], ones_d[:])
            ones_col = const_p.tile([P128, 1], RD)
            nc.sync.dma_start(ones_col[:], ones_d[0, :P128])
            mask_sb = const_p.tile([P128, NT], F32)
            for j in range(NT):
                nc.sync.dma_start(mask_sb[:, j:j + 1], mask_d[j, :])
            bq_sb = const_p.tile([P128, ET], F32)
            for j in range(ET):
                nc.sync.dma_start(bq_sb[:, j:j + 1], bq_d[j, :])

            # ---- persistent tensors ----
            wq_sb = [pers_p.tile([P128, E], RD, name=f"wq{i}") for i in range(ET)]
            wp_sb = [pers_p.tile([P128, E], RD, name=f"wp{i}") for i in range(ET)]
            wp_last = pers_p.tile([1, E], RD)
            for i in range(ET):
                nc.sync.dma_start(wq_sb[i][:], wqT_d[i * P128:(i + 1) * P128, :])
                nc.sync.dma_start(wp_sb[i][:], wpT_d[i * P128:(i + 1) * P128, :])
            nc.sync.dma_start(wp_last[:], wpT_d[E:E + 1, :])

            kT_sb = [pers_p.tile([P128, SP + P], RD, name=f"kT{i}") for i in range(ET)]
            v_sb = [pers_p.tile([P128, VA], RD, name=f"vsb{i}") for i in range(NT)]
            imp_sb = pers_p.tile([P128, T // P128], F32)

            # ================= Phase A: kv projection (sharded) ===========
            with ExitStack() as actx:
                akvc_p = actx.enter_context(tc.tile_pool(name="akvc", bufs=KCT + 1))
                aw_p = actx.enter_context(tc.tile_pool(name="aw", bufs=3))
                aps_p = actx.enter_context(
                    tc.tile_pool(name="aps", bufs=8, space="PSUM"))
                asb_p = actx.enter_context(tc.tile_pool(name="asb", bufs=4))

                kvc = []
                for kc in range(KCT + 1):
                    pr = P128 if kc < KCT else 1
                    t = akvc_p.tile([pr, SP + P], RD, name=f"kvc{kc}", tag="kvc")
                    nc.sync.dma_start(t[:], kvcT_d[kc * P128:kc * P128 + pr, :])
                    kvc.append(t)

                kT_in = dram_p.tile([E, SP + P], F32)
                v_in = dram_p.tile([SP, VA], F32)
                if use_cc:
                    kT_out = dram_p.tile([E, SP + P], F32)
                    v_out = dram_p.tile([SP, VA], F32)
                else:
                    kT_out, v_out = kT_in, v_in

                # A1: kT[o, n] = sum_i wkT[i, o] * kvcT[i, n]  (segment cols)
                kps = [aps_p.tile([P128, SP], F32, name=f"kps{o}", tag="ps")
                       for o in range(ET)]
                for kc in range(KCT + 1):
                    pr = P128 if kc < KCT else 1
                    wk = aw_p.tile([pr, E], RD, name="wk", tag="wk")
                    nc.sync.dma_start(wk[:], wkT_d[kc * P128:kc * P128 + pr, :])
                    for o in range(ET):
                        nc.tensor.matmul(
                            kps[o][:], wk[:, o * P128:(o + 1) * P128],
                            kvc[kc][:, :SP], start=(kc == 0), stop=(kc == KCT))
                # A1b: probe cols use the probe weight (K for batch0, V for batch1)
                pps = [aps_p.tile([P128, P], F32, name=f"pps{o}", tag="ps")
                       for o in range(ET)]
                for kc in range(KCT + 1):
                    pr = P128 if kc < KCT else 1
                    wpk = aw_p.tile([pr, E], RD, name="wpk", tag="wpk")
                    nc.sync.dma_start(wpk[:], wpkT_d[kc * P128:kc * P128 + pr, :])
                    for o in range(ET):
                        nc.tensor.matmul(
                            pps[o][:], wpk[:, o * P128:(o + 1) * P128],
                            kvc[kc][:, SP:SP + P], start=(kc == 0),
                            stop=(kc == KCT))
                for o in range(ET):
                    ksb = asb_p.tile([P128, SP + P], F32, name="ksb", tag="ksb")
                    nc.vector.tensor_copy(ksb[:, :SP], kps[o][:])
                    nc.vector.tensor_copy(ksb[:, SP:], pps[o][:])
                    nc.scalar.dma_start(kT_in[o * P128:(o + 1) * P128, :], ksb[:])

                # A2: v[n, j] = sum_i kvcT[i, n] * wvT[i, j]
                vps = [aps_p.tile([P128, VA // 2], F32, name=f"vps{i}", tag="ps")
                       for i in range(NT * 2)]
                for kc in range(KCT + 1):
                    pr = P128 if kc < KCT else 1
                    wv = aw_p.tile([pr, VA], RD, name="wv", tag="wv")
                    nc.sync.dma_start(wv[:], wvT_d[kc * P128:kc * P128 + pr, :])
                    for n in range(NT):
                        for hf in range(2):
                            nc.tensor.matmul(
                                vps[n * 2 + hf][:],
                                kvc[kc][:, n * P128:(n + 1) * P128],
                                wv[:, hf * (VA // 2):(hf + 1) * (VA // 2)],
                                start=(kc == 0), stop=(kc == KCT))
                for n in range(NT):
                    vsb = asb_p.tile([P128, VA], F32, name="vsb", tag="vsb")
                    for hf in range(2):
                        nc.vector.tensor_copy(
                            vsb[:, hf * (VA // 2):(hf + 1) * (VA // 2)],
                            vps[n * 2 + hf][:])
                    nc.scalar.dma_start(v_in[n * P128:(n + 1) * P128, :], vsb[:])

                if use_cc:
                    nc.gpsimd.collective_compute(
                        "AllReduce", mybir.AluOpType.add, replica_groups=groups,
                        ins=[kT_in[:]], outs=[kT_out[:]])
                    nc.gpsimd.collective_compute(
                        "AllReduce", mybir.AluOpType.add, replica_groups=groups,
                        ins=[v_in[:]], outs=[v_out[:]])

                for o in range(ET):
                    nc.sync.dma_start(
                        kT_sb[o][:], kT_out[o * P128:(o + 1) * P128, :].bitcast(RD))
                for n in range(NT):
                    nc.sync.dma_start(
                        v_sb[n][:], v_out[n * P128:(n + 1) * P128, :].bitcast(RD))

            # ================= Phase B: main pipeline =====================
            qt_p = ctx.enter_context(tc.tile_pool(name="qt", bufs=2))
            qp_p = ctx.enter_context(tc.tile_pool(name="qp", bufs=B_qp))
            pt_p = ctx.enter_context(tc.tile_pool(name="pt", bufs=B_pt))
            ot_p = ctx.enter_context(tc.tile_pool(name="ot", bufs=B_ot))
            sm_p = ctx.enter_context(tc.tile_pool(name="sm", bufs=3))
            pr_p = ctx.enter_context(tc.tile_pool(name="pr", bufs=3))
            ysb_p = ctx.enter_context(tc.tile_pool(name="ysb", bufs=B_ysb))
            ps_p = ctx.enter_context(tc.tile_pool(name="ps", bufs=B_ps, space="PSUM"))
            op_p = ctx.enter_context(tc.tile_pool(name="op", bufs=B_op, space="PSUM"))

            qpTs = {}
            for tcn in range(TC):
                t0 = tcn * 512
                qT = [qt_p.tile([P128, 512], RD, name=f"qT{i}", tag=f"qT{i}")
                      for i in range(ET)]
                for i in range(ET):
                    nc.sync.dma_start(qT[i][:],
                                      qT_d[i * P128:(i + 1) * P128, t0:t0 + 512])
                qpT = [qp_p.tile([P128, 512], RD, name=f"qpT{i}", tag=f"qpT{i}")
                       for i in range(ET)]
                qpTs[tcn] = qpT
                for o in range(ET):
                    qps = ps_p.tile([P128, 512], F32, name="qps", tag="ps")
                    for ic in range(ET):
                        nc.tensor.matmul(
                            qps[:], wq_sb[ic][:, o * P128:(o + 1) * P128],
                            qT[ic][:], start=(ic == 0), stop=(ic == ET - 1))
                    nc.scalar.activation(
                        qpT[o][:], qps[:], mybir.ActivationFunctionType.Identity,
                        bias=bq_sb[:, o:o + 1])

            for tcn in range(TC):
                t0 = tcn * 512
                qpT = qpTs[tcn]
                # ---- attention: logits+exp+denominators, then AV ----
                outT = [ot_p.tile([P128, 512], RD, name=f"outT{i}", tag=f"outT{i}")
                        for i in range(ET)]
                for h in range(H):
                    et, ro = (h * D) // P128, (h * D) % P128
                    qph = qpT[et][ro:ro + D, :]
                    PT = [pt_p.tile([P128, 512], RD, name=f"PT{n}",
                                    tag=f"PT{n}") for n in range(NT)]
                    for n in range(NT):
                        lt = ps_p.tile([P128, 512], F32, name="lt", tag="ps")
                        nc.tensor.matmul(
                            lt[:],
                            kT_sb[et][ro:ro + D, n * P128:(n + 1) * P128],
                            qph, start=True, stop=True)
                        nc.scalar.activation(
                            PT[n][:], lt[:],
                            mybir.ActivationFunctionType.Exp,
                            bias=mask_sb[:, n:n + 1], scale=float(scale))
                    op = op_p.tile([HD1, 512], F32, name="op", tag="op")
                    for n in range(NT):
                        nc.tensor.matmul(
                            op[:], v_sb[n][:, h * HD1:(h + 1) * HD1],
                            PT[n][:], start=(n == 0), stop=(n == NT - 1))
                    rden = sm_p.tile([1, 512], F32, name="rden", tag="rden")
                    nc.vector.reciprocal(rden[:], op[D:D + 1, :])
                    rbc = sm_p.tile([D, 512], F32, name="rbc", tag="rbc")
                    nc.gpsimd.partition_broadcast(rbc[:], rden[:])
                    nc.vector.tensor_mul(outT[et][ro:ro + D, :],
                                         op[0:D, :], rbc[:])

                # ---- probes: logits [t, 4, P] per head, batched stats ----
                maxPH = pr_p.tile([P128, 4, H], F32, name="maxPH", tag="maxPH")
                denPH = pr_p.tile([P128, 4, H], F32, name="denPH", tag="denPH")
                for h in range(H):
                    et, ro = (h * D) // P128, (h * D) % P128
                    pl = ps_p.tile([P128, 4, P], F32, name="pl", tag="ps")
                    for tt in range(4):
                        nc.tensor.matmul(
                            pl[:, tt, :],
                            qpT[et][ro:ro + D, tt * P128:(tt + 1) * P128],
                            kT_sb[et][ro:ro + D, SP:SP + P],
                            start=True, stop=True)
                    sp = pr_p.tile([P128, 4, P], F32, name="sp", tag="sp")
                    nc.scalar.activation(
                        sp[:], pl[:], mybir.ActivationFunctionType.Exp)
                    nc.vector.tensor_reduce(
                        maxPH[:, :, h], sp[:], axis=mybir.AxisListType.X,
                        op=mybir.AluOpType.max)
                    nc.vector.tensor_reduce(
                        denPH[:, :, h], sp[:], axis=mybir.AxisListType.X,
                        op=mybir.AluOpType.add)
                rcp = pr_p.tile([P128, 4, H], F32, name="rcp", tag="rcp")
                nc.vector.reciprocal(rcp[:], denPH[:])
                iv = pr_p.tile([P128, 4, H], F32, name="iv", tag="iv")
                nc.vector.tensor_mul(iv[:], maxPH[:], rcp[:])
                red = pr_p.tile([P128, 4], F32, name="red", tag="red")
                nc.vector.tensor_reduce(
                    red[:], iv[:], axis=mybir.AxisListType.X,
                    op=mybir.AluOpType.add)
                nc.vector.tensor_scalar_mul(
                    imp_sb[:, tcn * 4:tcn * 4 + 4], red[:], 1.0 / H)

                # ---- output projection ----
                for tt in range(4):
                    ysb = ysb_p.tile([P128, E], F32, name="ysb", tag="ysb")
                    for hf in range(2):
                        yps = ps_p.tile([P128, E // 2], F32, name="yps", tag="ps")
                        for ec in range(ET):
                            nc.tensor.matmul(
                                yps[:],
                                outT[ec][:, tt * P128:(tt + 1) * P128],
                                wp_sb[ec][:, hf * (E // 2):(hf + 1) * (E // 2)],
                                start=(ec == 0), stop=False)
                        nc.tensor.matmul(
                            yps[:], ones_row[:, tt * P128:(tt + 1) * P128],
                            wp_last[:, hf * (E // 2):(hf + 1) * (E // 2)],
                            start=False, stop=True)
                        nc.scalar.copy(ysb[:, hf * (E // 2):(hf + 1) * (E // 2)],
                                       yps[:])
                    nc.scalar.dma_start(
                        y_d[t0 + tt * P128:t0 + (tt + 1) * P128, :], ysb[:])

            for j in range(T // P128):
                nc.scalar.dma_start(imp_d[j, :], imp_sb[:, j:j + 1])

    nc.compile()
    return nc


def kernel(q, kv_compact, W_q, b_q, W_kv, v_bias, W_proj, b_proj,
           cu_seqlens_k, max_seqlen_k, probe_ids):
    q = np.asarray(q, np.float32)
    kv_compact = np.asarray(kv_compact, np.float32)
    W_q = np.asarray(W_q, np.float32)
    b_q = np.asarray(b_q, np.float32)
    W_kv = np.asarray(W_kv, np.float32)
    v_bias = np.asarray(v_bias, np.float32)
    W_proj = np.asarray(W_proj, np.float32)
    b_proj = np.asarray(b_proj, np.float32)
    cu = np.asarray(cu_seqlens_k, np.int64)
    probe_ids = np.asarray(probe_ids, np.int64)

    B, LQ, E = q.shape
    N, KVD = kv_compact.shape
    H = 12
    D = E // H
    assert B == 2 and NCORES % B == 0
    CPB = NCORES // B            # cores per batch
    T = (B * LQ) // NCORES       # tokens per core
    P = len(probe_ids)
    KC = KVD // CPB
    S = [int(cu[b + 1] - cu[b]) for b in range(B)]
    SP = max(P128, -(-max(S) // P128) * P128)

    # ---------- host-side input prep ----------
    kvT = np.ascontiguousarray(kv_compact.T)               # [KVD, N]
    probeT = kvT[:, probe_ids]                             # [KVD, P]
    wkT_full = np.ascontiguousarray(W_kv[:E].T)            # [KVD, E]
    wvT_full = np.ascontiguousarray(W_kv[E:].T)            # [KVD, E]
    wqT = np.ascontiguousarray(W_q.T)
    HD1 = D + 1
    VA = H * HD1
    wvT_aug = np.zeros((KVD + 1, VA), np.float32)
    for h in range(H):
        wvT_aug[:KVD, h * HD1:h * HD1 + D] = wvT_full[:, h * D:(h + 1) * D]
        wvT_aug[KVD, h * HD1 + D] = 1.0
    b_eff = W_proj @ v_bias + b_proj
    wpT = np.concatenate([np.ascontiguousarray(W_proj.T), b_eff[None, :]], 0)

    in_maps = []
    for c in range(NCORES):
        b, g = c // CPB, c % CPB
        lo, hi = int(cu[b]), int(cu[b + 1])
        segT = np.zeros((KVD, SP), np.float32)
        segT[:, :hi - lo] = kvT[:, lo:hi]
        kvcT_aug = np.concatenate([segT, probeT], 1)       # [KVD, SP+P]
        kvcT = np.zeros((KC + 1, SP + P), np.float32)
        kvcT[:KC] = kvcT_aug[g * KC:(g + 1) * KC]
        if g == 0:
            kvcT[KC] = 1.0    # ones row: v denominator + probe v_bias source
        wkT = np.zeros((KC + 1, E), np.float32)
        wkT[:KC] = wkT_full[g * KC:(g + 1) * KC]
        wvT = np.zeros((KC + 1, VA), np.float32)
        wvT[:KC] = wvT_aug[g * KC:(g + 1) * KC]
        if g == 0:
            wvT[KC] = wvT_aug[KVD]
        # probe projection: batch 0 probes against K, batch 1 against V(+bias)
        wpk_full = wkT_full if b == 0 else wvT_full
        wpkT = np.zeros((KC + 1, E), np.float32)
        wpkT[:KC] = wpk_full[g * KC:(g + 1) * KC]
        if g == 0 and b == 1:
            wpkT[KC] = v_bias
        mask = np.zeros((SP // P128, P128), np.float32)
        mask.reshape(-1)[hi - lo:] = -1e30
        in_maps.append({
            "qT_s": np.ascontiguousarray(q[b, g * T:(g + 1) * T].T),
            "kvcT": kvcT, "wkT": wkT, "wvT": wvT, "wpkT": wpkT,
            "wqT": wqT, "wpT": wpT, "maskns": mask,
            "bq": np.ascontiguousarray(b_q.reshape(E // P128, P128)),
            "ones": np.ones((1, 512), np.float32),
        })

    nc = _build(T, SP, P, KC, E, H, D, True)
    res = run_bass_kernel_spmd(nc, in_maps, core_ids=list(range(NCORES)))

    out = np.empty((B, LQ, E), np.float32)
    imp = np.empty((B, LQ), np.float32)
    for c in range(NCORES):
        b, g = c // CPB, c % CPB
        out[b, g * T:(g + 1) * T] = res.results[c]["y_s"]
        imp[b, g * T:(g + 1) * T] = res.results[c]["imp_s"].reshape(-1)
    return out, imp


# revision 19
# speedup vs baseline: 1.1288x; 1.0253x over previous
"""CrossAttnBlock Trainium2 kernel (8 NeuronCores, SPMD).

Sharding: query-sequence parallel. 16384 query rows (B*Lq) are split 8 ways;
cores 0-3 take batch 0, cores 4-7 take batch 1 (2048 rows each). The small
ragged KV (N=512) projection is contraction-sharded 4 ways inside each batch
group and AllReduce'd; each core only materializes its own batch's KV segment
plus the probe columns.

Layout plan (per core, T=2048 query tokens):
  qT   = host-pretransposed q shard               [E, T]    (f32r, direct DMA)
  qpT  = W_qT.T @ qT + b_q (per-partition add)    [E, T]    psum->sbuf
  kT   = kv-proj, k half, transposed layout       [E, S+P]  (AllReduce'd)
         probe cols use W_k (batch 0) / W_v + v_bias (batch 1) -- the
         reference's einsum 'bhqd,bhpd' binds batch to the k/v axis of kv.
  v    = kv-proj, v half, natural layout          [S, E]    (AllReduce'd)
  LT   = kT_h.T @ qpT_h per head                  [S, T]    psum (transposed)
  PT   = exp(scale*LT + mask_bias)                [S, T]    f32r sbuf
  den  = ones.T @ PT per head -> one [H, T] psum; single reciprocal
  oT   = v_h.T @ PT (unnormalized)                [D, T]    psum
  out  = oT * gpsimd-broadcast(1/den_h)           [E, T]    sbuf
  y    = outT.T @ W_projT (+b_eff via ones row)   [T, E]    psum->sbuf->DRAM
  probes: natural-layout logits [T, 4, P] per head, one exp per head-chunk,
          batched free-dim max/sum reduces -> imp = mean_h(max/den).

v_bias enters the attention output only through b_eff = W_proj@v_bias + b_proj
(softmax rows sum to 1), applied via an appended ones row on the proj
stationary. All PE-facing tensors are float32r (fp32 bits, reduced-precision
full-rate streaming); PSUM accumulation stays fp32.
"""

import functools
import sys

import numpy as np

try:
    import concourse.bass as bass  # noqa: F401
except ImportError:
    sys.path.insert(0, "/opt/trn_rl_repo")

import concourse.bass as bass
import concourse.tile as tile
from concourse import bacc, mybir
from concourse.bass_utils import run_bass_kernel_spmd

F32 = mybir.dt.float32
F32R = mybir.dt.float32r

NCORES = 8
P128 = 128


@functools.lru_cache(maxsize=4)
def _build(T, SP, P, KC, E, H, D, use_f32r, use_cc=True, bufs_cfg=()):
    """Build + compile the per-core Bass module. All shapes are compile-time.

    T  : query tokens per core (2048)
    SP : padded KV segment length (mult of 128)
    P  : number of probe ids
    KC : contraction rows of the kv projection handled per core (KVD/4)
    """
    assert T % 512 == 0 and SP % P128 == 0 and E % P128 == 0
    HD1 = D + 1         # per-head v width incl. ones column
    VA = H * HD1        # 780
    NT = SP // P128     # kv n-tiles (2)
    ET = E // P128      # feature tiles (6)
    TC = T // 512       # 512-token chunks (4)
    KCT = KC // P128    # kv-proj K tiles per core (8) + 1 single-row
    scale = 1.0 / np.sqrt(D)
    RD = F32R if use_f32r else F32
    cfg = dict(bufs_cfg)
    B_qp = cfg.get("qp", 5); B_pt = cfg.get("pt", 3); B_ot = cfg.get("ot", 2)
    B_ps = cfg.get("ps", 5); B_op = cfg.get("op", 3); B_ysb = cfg.get("ysb", 3)
    HB = cfg.get("HB", 4)

    nc = bacc.Bacc("TRN2", target_bir_lowering=False, debug=False,
                   num_devices=NCORES)

    qT_d = nc.dram_tensor("qT_s", [E, T], RD, kind="ExternalInput").ap()
    kvcT_d = nc.dram_tensor("kvcT", [KC + 1, SP + P], RD, kind="ExternalInput").ap()
    wkT_d = nc.dram_tensor("wkT", [KC + 1, E], RD, kind="ExternalInput").ap()
    wvT_d = nc.dram_tensor("wvT", [KC + 1, VA], RD, kind="ExternalInput").ap()
    wpkT_d = nc.dram_tensor("wpkT", [KC + 1, E], RD, kind="ExternalInput").ap()
    wqT_d = nc.dram_tensor("wqT", [E, E], RD, kind="ExternalInput").ap()
    wpT_d = nc.dram_tensor("wpT", [E + 1, E], RD, kind="ExternalInput").ap()
    bq_d = nc.dram_tensor("bq", [E // P128, P128], F32, kind="ExternalInput").ap()
    mask_d = nc.dram_tensor("maskns", [SP // P128, P128], F32,
                            kind="ExternalInput").ap()
    ones_d = nc.dram_tensor("ones", [1, 512], RD, kind="ExternalInput").ap()
    y_d = nc.dram_tensor("y_s", [T, E], F32, kind="ExternalOutput").ap()
    imp_d = nc.dram_tensor("imp_s", [T // P128, P128], F32, kind="ExternalOutput").ap()

    groups = [[0, 1, 2, 3], [4, 5, 6, 7]]

    with tile.TileContext(nc) as tc:
        from contextlib import ExitStack
        with ExitStack() as ctx:
            const_p = ctx.enter_context(tc.tile_pool(name="const", bufs=1))
            pers_p = ctx.enter_context(tc.tile_pool(name="pers", bufs=1))
            dram_p = ctx.enter_context(tc.tile_pool(name="dram", bufs=1, space="DRAM"))

            ones_row = const_p.tile([1, 512], RD)
            nc.sync.dma_start(ones_row[:], ones_d[:])
            ones_col = const_p.tile([P128, 1], RD)
            nc.sync.dma_start(ones_col[:], ones_d[0, :P128])
            mask_sb = const_p.tile([P128, NT], F32)
            for j in range(NT):
                nc.sync.dma_start(mask_sb[:, j:j + 1], mask_d[j, :])
            bq_sb = const_p.tile([P128, ET], F32)
            for j in range(ET):
                nc.sync.dma_start(bq_sb[:, j:j + 1], bq_d[j, :])

            # ---- persistent tensors ----
            wq_sb = [pers_p.tile([P128, E], RD, name=f"wq{i}") for i in range(ET)]
            wp_sb = [pers_p.tile([P128, E], RD, name=f"wp{i}") for i in range(ET)]
            wp_last = pers_p.tile([1, E], RD)
            for i in range(ET):
                nc.sync.dma_start(wq_sb[i][:], wqT_d[i * P128:(i + 1) * P128, :])

            kT_sb = [pers_p.tile([P128, SP + P], RD, name=f"kT{i}") for i in range(ET)]
            v_sb = [pers_p.tile([P128, VA], RD, name=f"vsb{i}") for i in range(NT)]
            imp_sb = pers_p.tile([P128, T // P128], F32)

            # ================= Phase A: kv projection (sharded) ===========
            with ExitStack() as actx:
                akvc_p = actx.enter_context(tc.tile_pool(name="akvc", bufs=KCT + 1))
                aw_p = actx.enter_context(tc.tile_pool(name="aw", bufs=3))
                aps_p = actx.enter_context(
                    tc.tile_pool(name="aps", bufs=8, space="PSUM"))
                asb_p = actx.enter_context(tc.tile_pool(name="asb", bufs=4))

                kvc = []
                for kc in range(KCT + 1):
                    pr = P128 if kc < KCT else 1
                    t = akvc_p.tile([pr, SP + P], RD, name=f"kvc{kc}", tag="kvc")
                    nc.sync.dma_start(t[:], kvcT_d[kc * P128:kc * P128 + pr, :])
                    kvc.append(t)

                kT_in = dram_p.tile([E, SP + P], F32)
                v_in = dram_p.tile([SP, VA], F32)
                if use_cc:
                    kT_out = dram_p.tile([E, SP + P], F32)
                    v_out = dram_p.tile([SP, VA], F32)
                else:
                    kT_out, v_out = kT_in, v_in

                # A1: kT[o, n] = sum_i wkT[i, o] * kvcT[i, n]  (segment cols)
                kps = [aps_p.tile([P128, SP], F32, name=f"kps{o}", tag="ps")
                       for o in range(ET)]
                for kc in range(KCT + 1):
                    pr = P128 if kc < KCT else 1
                    wk = aw_p.tile([pr, E], RD, name="wk", tag="wk")
                    nc.sync.dma_start(wk[:], wkT_d[kc * P128:kc * P128 + pr, :])
                    for o in range(ET):
                        nc.tensor.matmul(
                            kps[o][:], wk[:, o * P128:(o + 1) * P128],
                            kvc[kc][:, :SP], start=(kc == 0), stop=(kc == KCT))
                # A1b: probe cols use the probe weight (K for batch0, V for batch1)
                pps = [aps_p.tile([P128, P], F32, name=f"pps{o}", tag="ps")
                       for o in range(ET)]
                for kc in range(KCT + 1):
                    pr = P128 if kc < KCT else 1
                    wpk = aw_p.tile([pr, E], RD, name="wpk", tag="wpk")
                    nc.sync.dma_start(wpk[:], wpkT_d[kc * P128:kc * P128 + pr, :])
                    for o in range(ET):
                        nc.tensor.matmul(
                            pps[o][:], wpk[:, o * P128:(o + 1) * P128],
                            kvc[kc][:, SP:SP + P], start=(kc == 0),
                            stop=(kc == KCT))
                for o in range(ET):
                    ksb = asb_p.tile([P128, SP + P], F32, name="ksb", tag="ksb")
                    nc.vector.tensor_copy(ksb[:, :SP], kps[o][:])
                    nc.vector.tensor_copy(ksb[:, SP:], pps[o][:])
                    nc.scalar.dma_start(kT_in[o * P128:(o + 1) * P128, :], ksb[:])

                # A2: v[n, j] = sum_i kvcT[i, n] * wvT[i, j]
                vps = [aps_p.tile([P128, VA // 2], F32, name=f"vps{i}", tag="ps")
                       for i in range(NT * 2)]
                for kc in range(KCT + 1):
                    pr = P128 if kc < KCT else 1
                    wv = aw_p.tile([pr, VA], RD, name="wv", tag="wv")
                    nc.sync.dma_start(wv[:], wvT_d[kc * P128:kc * P128 + pr, :])
                    for n in range(NT):
                        for hf in range(2):
                            nc.tensor.matmul(
                                vps[n * 2 + hf][:],
                                kvc[kc][:, n * P128:(n + 1) * P128],
                                wv[:, hf * (VA // 2):(hf + 1) * (VA // 2)],
                                start=(kc == 0), stop=(kc == KCT))
                for n in range(NT):
                    vsb = asb_p.tile([P128, VA], F32, name="vsb", tag="vsb")
                    for hf in range(2):
                        nc.vector.tensor_copy(
                            vsb[:, hf * (VA // 2):(hf + 1) * (VA // 2)],
                            vps[n * 2 + hf][:])
                    nc.scalar.dma_start(v_in[n * P128:(n + 1) * P128, :], vsb[:])

                if use_cc:
                    nc.gpsimd.collective_compute(
                        "AllReduce", mybir.AluOpType.add, replica_groups=groups,
                        ins=[kT_in[:]], outs=[kT_out[:]])
                    nc.gpsimd.collective_compute(
                        "AllReduce", mybir.AluOpType.add, replica_groups=groups,
                        ins=[v_in[:]], outs=[v_out[:]])

                for o in range(ET):
                    nc.sync.dma_start(
                        kT_sb[o][:], kT_out[o * P128:(o + 1) * P128, :].bitcast(RD))
                for n in range(NT):
                    nc.sync.dma_start(
                        v_sb[n][:], v_out[n * P128:(n + 1) * P128, :].bitcast(RD))

            for i in range(ET):
                nc.scalar.dma_start(wp_sb[i][:], wpT_d[i * P128:(i + 1) * P128, :])
            nc.scalar.dma_start(wp_last[:], wpT_d[E:E + 1, :])

            # ================= Phase B: main pipeline =====================
            qt_p = ctx.enter_context(tc.tile_pool(name="qt", bufs=2))
            qp_p = ctx.enter_context(tc.tile_pool(name="qp", bufs=B_qp))
            pt_p = ctx.enter_context(tc.tile_pool(name="pt", bufs=B_pt))
            ot_p = ctx.enter_context(tc.tile_pool(name="ot", bufs=B_ot))
            sm_p = ctx.enter_context(tc.tile_pool(name="sm", bufs=3))
            pr_p = ctx.enter_context(tc.tile_pool(name="pr", bufs=3))
            ysb_p = ctx.enter_context(tc.tile_pool(name="ysb", bufs=B_ysb))
            ps_p = ctx.enter_context(tc.tile_pool(name="ps", bufs=B_ps, space="PSUM"))
            op_p = ctx.enter_context(tc.tile_pool(name="op", bufs=B_op, space="PSUM"))

            qpTs = {}
            for tcn in range(TC):
                t0 = tcn * 512
                qT = [qt_p.tile([P128, 512], RD, name=f"qT{i}", tag=f"qT{i}")
                      for i in range(ET)]
                for i in range(ET):
                    nc.sync.dma_start(qT[i][:],
                                      qT_d[i * P128:(i + 1) * P128, t0:t0 + 512])
                qpT = [qp_p.tile([P128, 512], RD, name=f"qpT{i}", tag=f"qpT{i}")
                       for i in range(ET)]
                qpTs[tcn] = qpT
                for o in range(ET):
                    qps = ps_p.tile([P128, 512], F32, name="qps", tag="ps")
                    for ic in range(ET):
                        nc.tensor.matmul(
                            qps[:], wq_sb[ic][:, o * P128:(o + 1) * P128],
                            qT[ic][:], start=(ic == 0), stop=(ic == ET - 1))
                    nc.scalar.activation(
                        qpT[o][:], qps[:], mybir.ActivationFunctionType.Identity,
                        bias=bq_sb[:, o:o + 1])

            for tcn in range(TC):
                t0 = tcn * 512
                qpT = qpTs[tcn]
                # ---- attention: logits+exp+denominators, then AV ----
                outT = [ot_p.tile([P128, 512], RD, name=f"outT{i}", tag=f"outT{i}")
                        for i in range(ET)]
                for h in range(H):
                    et, ro = (h * D) // P128, (h * D) % P128
                    qph = qpT[et][ro:ro + D, :]
                    PT = [pt_p.tile([P128, 512], RD, name=f"PT{n}",
                                    tag=f"PT{n}") for n in range(NT)]
                    for n in range(NT):
                        lt = ps_p.tile([P128, 512], F32, name="lt", tag="ps")
                        nc.tensor.matmul(
                            lt[:],
                            kT_sb[et][ro:ro + D, n * P128:(n + 1) * P128],
                            qph, start=True, stop=True)
                        nc.scalar.activation(
                            PT[n][:], lt[:],
                            mybir.ActivationFunctionType.Exp,
                            bias=mask_sb[:, n:n + 1], scale=float(scale))
                    op = op_p.tile([HD1, 512], F32, name="op", tag="op")
                    for n in range(NT):
                        nc.tensor.matmul(
                            op[:], v_sb[n][:, h * HD1:(h + 1) * HD1],
                            PT[n][:], start=(n == 0), stop=(n == NT - 1))
                    rden = sm_p.tile([1, 512], F32, name="rden", tag="rden")
                    nc.vector.reciprocal(rden[:], op[D:D + 1, :])
                    rbc = sm_p.tile([D, 512], F32, name="rbc", tag="rbc")
                    nc.gpsimd.partition_broadcast(rbc[:], rden[:])
                    nc.vector.tensor_mul(outT[et][ro:ro + D, :],
                                         op[0:D, :], rbc[:])

                # ---- probes: logits [t, 4, P] per head, batched stats ----
                maxPH = pr_p.tile([P128, 4, H], F32, name="maxPH", tag="maxPH")
                denPH = pr_p.tile([P128, 4, H], F32, name="denPH", tag="denPH")
                for h in range(H):
                    et, ro = (h * D) // P128, (h * D) % P128
                    pl = ps_p.tile([P128, 4, P], F32, name="pl", tag="ps")
                    for tt in range(4):
                        nc.tensor.matmul(
                            pl[:, tt, :],
                            qpT[et][ro:ro + D, tt * P128:(tt + 1) * P128],
                            kT_sb[et][ro:ro + D, SP:SP + P],
                            start=True, stop=True)
                    sp = pr_p.tile([P128, 4, P], F32, name="sp", tag="sp")
                    nc.scalar.activation(
                        sp[:], pl[:], mybir.ActivationFunctionType.Exp)
                    nc.vector.tensor_reduce(
                        maxPH[:, :, h], sp[:], axis=mybir.AxisListType.X,
                        op=mybir.AluOpType.max)
                    nc.vector.tensor_reduce(
                        denPH[:, :, h], sp[:], axis=mybir.AxisListType.X,
                        op=mybir.AluOpType.add)
                rcp = pr_p.tile([P128, 4, H], F32, name="rcp", tag="rcp")
                nc.vector.reciprocal(rcp[:], denPH[:])
                iv = pr_p.tile([P128, 4, H], F32, name="iv", tag="iv")
                nc.vector.tensor_mul(iv[:], maxPH[:], rcp[:])
                red = pr_p.tile([P128, 4], F32, name="red", tag="red")
                nc.vector.tensor_reduce(
                    red[:], iv[:], axis=mybir.AxisListType.X,
                    op=mybir.AluOpType.add)
                nc.vector.tensor_scalar_mul(
                    imp_sb[:, tcn * 4:tcn * 4 + 4], red[:], 1.0 / H)

                # ---- output projection ----
                for tt in range(4):
                    ysb = ysb_p.tile([P128, E], F32, name="ysb", tag="ysb")
                    for hf in range(2):
                        yps = ps_p.tile([P128, E // 2], F32, name="yps", tag="ps")
                        for ec in range(ET):
                            nc.tensor.matmul(
                                yps[:],
                                outT[ec][:, tt * P128:(tt + 1) * P128],
                                wp_sb[ec][:, hf * (E // 2):(hf + 1) * (E // 2)],
                                start=(ec == 0), stop=False)
                        nc.tensor.matmul(
                            yps[:], ones_row[:, tt * P128:(tt + 1) * P128],
                            wp_last[:, hf * (E // 2):(hf + 1) * (E // 2)],
                            start=False, stop=True)
                        nc.scalar.copy(ysb[:, hf * (E // 2):(hf + 1) * (E // 2)],
                                       yps[:])
                    nc.scalar.dma_start(
                        y_d[t0 + tt * P128:t0 + (tt + 1) * P128, :], ysb[:])

            for j in range(T // P128):
                nc.scalar.dma_start(imp_d[j, :], imp_sb[:, j:j + 1])

    nc.compile()
    return nc


def kernel(q, kv_compact, W_q, b_q, W_kv, v_bias, W_proj, b_proj,
           cu_seqlens_k, max_seqlen_k, probe_ids):
    q = np.asarray(q, np.float32)
    kv_compact = np.asarray(kv_compact, np.float32)
    W_q = np.asarray(W_q, np.float32)
    b_q = np.asarray(b_q, np.float32)
    W_kv = np.asarray(W_kv, np.float32)
    v_bias = np.asarray(v_bias, np.float32)
    W_proj = np.asarray(W_proj, np.float32)
    b_proj = np.asarray(b_proj, np.float32)
    cu = np.asarray(cu_seqlens_k, np.int64)
    probe_ids = np.asarray(probe_ids, np.int64)

    B, LQ, E = q.shape
    N, KVD = kv_compact.shape
    H = 12
    D = E // H
    assert B == 2 and NCORES % B == 0
    CPB = NCORES // B            # cores per batch
    T = (B * LQ) // NCORES       # tokens per core
    P = len(probe_ids)
    KC = KVD // CPB
    S = [int(cu[b + 1] - cu[b]) for b in range(B)]
    SP = max(P128, -(-max(S) // P128) * P128)

    # ---------- host-side input prep ----------
    kvT = np.ascontiguousarray(kv_compact.T)               # [KVD, N]
    probeT = kvT[:, probe_ids]                             # [KVD, P]
    wkT_full = np.ascontiguousarray(W_kv[:E].T)            # [KVD, E]
    wvT_full = np.ascontiguousarray(W_kv[E:].T)            # [KVD, E]
    wqT = np.ascontiguousarray(W_q.T)
    HD1 = D + 1
    VA = H * HD1
    wvT_aug = np.zeros((KVD + 1, VA), np.float32)
    for h in range(H):
        wvT_aug[:KVD, h * HD1:h * HD1 + D] = wvT_full[:, h * D:(h + 1) * D]
        wvT_aug[KVD, h * HD1 + D] = 1.0
    b_eff = W_proj @ v_bias + b_proj
    wpT = np.concatenate([np.ascontiguousarray(W_proj.T), b_eff[None, :]], 0)

    in_maps = []
    for c in range(NCORES):
        b, g = c // CPB, c % CPB
        lo, hi = int(cu[b]), int(cu[b + 1])
        segT = np.zeros((KVD, SP), np.float32)
        segT[:, :hi - lo] = kvT[:, lo:hi]
        kvcT_aug = np.concatenate([segT, probeT], 1)       # [KVD, SP+P]
        kvcT = np.zeros((KC + 1, SP + P), np.float32)
        kvcT[:KC] = kvcT_aug[g * KC:(g + 1) * KC]
        if g == 0:
            kvcT[KC] = 1.0    # ones row: v denominator + probe v_bias source
        wkT = np.zeros((KC + 1, E), np.float32)
        wkT[:KC] = wkT_full[g * KC:(g + 1) * KC]
        wvT = np.zeros((KC + 1, VA), np.float32)
        wvT[:KC] = wvT_aug[g * KC:(g + 1) * KC]
        if g == 0:
            wvT[KC] = wvT_aug[KVD]
        # probe projection: batch 0 probes against K, batch 1 against V(+bias)
        wpk_full = wkT_full if b == 0 else wvT_full
        wpkT = np.zeros((KC + 1, E), np.float32)
        wpkT[:KC] = wpk_full[g * KC:(g + 1) * KC]
        if g == 0 and b == 1:
            wpkT[KC] = v_bias
        mask = np.zeros((SP // P128, P128), np.float32)
        mask.reshape(-1)[hi - lo:] = -1e30
        in_maps.append({
            "qT_s": np.ascontiguousarray(q[b, g * T:(g + 1) * T].T),
            "kvcT": kvcT, "wkT": wkT, "wvT": wvT, "wpkT": wpkT,
            "wqT": wqT, "wpT": wpT, "maskns": mask,
            "bq": np.ascontiguousarray(b_q.reshape(E // P128, P128)),
            "ones": np.ones((1, 512), np.float32),
        })

    nc = _build(T, SP, P, KC, E, H, D, True)
    res = run_bass_kernel_spmd(nc, in_maps, core_ids=list(range(NCORES)))

    out = np.empty((B, LQ, E), np.float32)
    imp = np.empty((B, LQ), np.float32)
    for c in range(NCORES):
        b, g = c // CPB, c % CPB
        out[b, g * T:(g + 1) * T] = res.results[c]["y_s"]
        imp[b, g * T:(g + 1) * T] = res.results[c]["imp_s"].reshape(-1)
    return out, imp


# revision 21
# speedup vs baseline: 1.1474x; 1.0164x over previous
"""CrossAttnBlock Trainium2 kernel (8 NeuronCores, SPMD).

Sharding: query-sequence parallel. 16384 query rows (B*Lq) are split 8 ways;
cores 0-3 take batch 0, cores 4-7 take batch 1 (2048 rows each). The small
ragged KV (N=512) projection is contraction-sharded 4 ways inside each batch
group and AllReduce'd; each core only materializes its own batch's KV segment
plus the probe columns.

Layout plan (per core, T=2048 query tokens):
  qT   = host-pretransposed q shard               [E, T]    (f32r, direct DMA)
  qpT  = W_qT.T @ qT + b_q (per-partition add)    [E, T]    psum->sbuf
  kT   = kv-proj, k half, transposed layout       [E, S+P]  (AllReduce'd)
         probe cols use W_k (batch 0) / W_v + v_bias (batch 1) -- the
         reference's einsum 'bhqd,bhpd' binds batch to the k/v axis of kv.
  v    = kv-proj, v half, natural layout          [S, E]    (AllReduce'd)
  LT   = kT_h.T @ qpT_h per head                  [S, T]    psum (transposed)
  PT   = exp(scale*LT + mask_bias)                [S, T]    f32r sbuf
  den  = ones.T @ PT per head -> one [H, T] psum; single reciprocal
  oT   = v_h.T @ PT (unnormalized)                [D, T]    psum
  out  = oT * gpsimd-broadcast(1/den_h)           [E, T]    sbuf
  y    = outT.T @ W_projT (+b_eff via ones row)   [T, E]    psum->sbuf->DRAM
  probes: natural-layout logits [T, 4, P] per head, one exp per head-chunk,
          batched free-dim max/sum reduces -> imp = mean_h(max/den).

v_bias enters the attention output only through b_eff = W_proj@v_bias + b_proj
(softmax rows sum to 1), applied via an appended ones row on the proj
stationary. All PE-facing tensors are float32r (fp32 bits, reduced-precision
full-rate streaming); PSUM accumulation stays fp32.
"""

import functools
import sys

import numpy as np

try:
    import concourse.bass as bass  # noqa: F401
except ImportError:
    sys.path.insert(0, "/opt/trn_rl_repo")

import concourse.bass as bass
import concourse.tile as tile
from concourse import bacc, mybir
from concourse.bass_utils import run_bass_kernel_spmd

F32 = mybir.dt.float32
F32R = mybir.dt.float32r

NCORES = 8
P128 = 128


@functools.lru_cache(maxsize=4)
def _build(T, SP, P, KC, E, H, D, use_f32r, use_cc=True, bufs_cfg=()):
    """Build + compile the per-core Bass module. All shapes are compile-time.

    T  : query tokens per core (2048)
    SP : padded KV segment length (mult of 128)
    P  : number of probe ids
    KC : contraction rows of the kv projection handled per core (KVD/4)
    """
    assert T % 512 == 0 and SP % P128 == 0 and E % P128 == 0
    HD1 = D + 1         # per-head v width incl. ones column
    VA = H * HD1        # 780
    NT = SP // P128     # kv n-tiles (2)
    ET = E // P128      # feature tiles (6)
    TC = T // 512       # 512-token chunks (4)
    KCT = KC // P128    # kv-proj K tiles per core (8) + 1 single-row
    scale = 1.0 / np.sqrt(D)
    RD = F32R if use_f32r else F32
    cfg = dict(bufs_cfg)
    B_qp = cfg.get("qp", 5); B_pt = cfg.get("pt", 3); B_ot = cfg.get("ot", 2)
    B_ps = cfg.get("ps", 6); B_op = cfg.get("op", 2); B_ysb = cfg.get("ysb", 3)
    HB = cfg.get("HB", 4)

    nc = bacc.Bacc("TRN2", target_bir_lowering=False, debug=False,
                   num_devices=NCORES)

    qT_d = nc.dram_tensor("qT_s", [E, T], RD, kind="ExternalInput").ap()
    kvcT_d = nc.dram_tensor("kvcT", [KC + 1, SP + P], RD, kind="ExternalInput").ap()
    wkT_d = nc.dram_tensor("wkT", [KC + 1, E], RD, kind="ExternalInput").ap()
    wvT_d = nc.dram_tensor("wvT", [KC + 1, VA], RD, kind="ExternalInput").ap()
    wpkT_d = nc.dram_tensor("wpkT", [KC + 1, E], RD, kind="ExternalInput").ap()
    wqT_d = nc.dram_tensor("wqT", [E, E], RD, kind="ExternalInput").ap()
    wpT_d = nc.dram_tensor("wpT", [E + 1, E], RD, kind="ExternalInput").ap()
    bq_d = nc.dram_tensor("bq", [E // P128, P128], F32, kind="ExternalInput").ap()
    mask_d = nc.dram_tensor("maskns", [SP // P128, P128], F32,
                            kind="ExternalInput").ap()
    ones_d = nc.dram_tensor("ones", [1, 512], RD, kind="ExternalInput").ap()
    y_d = nc.dram_tensor("y_s", [T, E], F32, kind="ExternalOutput").ap()
    imp_d = nc.dram_tensor("imp_s", [T // P128, P128], F32, kind="ExternalOutput").ap()

    groups = [[0, 1, 2, 3], [4, 5, 6, 7]]

    with tile.TileContext(nc) as tc:
        from contextlib import ExitStack
        with ExitStack() as ctx:
            const_p = ctx.enter_context(tc.tile_pool(name="const", bufs=1))
            pers_p = ctx.enter_context(tc.tile_pool(name="pers", bufs=1))
            dram_p = ctx.enter_context(tc.tile_pool(name="dram", bufs=1, space="DRAM"))

            ones_row = const_p.tile([1, 512], RD)
            nc.sync.dma_start(ones_row[:], ones_d[:])
            ones_col = const_p.tile([P128, 1], RD)
            nc.sync.dma_start(ones_col[:], ones_d[0, :P128])
            mask_sb = const_p.tile([P128, NT], F32)
            for j in range(NT):
                nc.sync.dma_start(mask_sb[:, j:j + 1], mask_d[j, :])
            bq_sb = const_p.tile([P128, ET], F32)
            for j in range(ET):
                nc.sync.dma_start(bq_sb[:, j:j + 1], bq_d[j, :])

            # ---- persistent tensors ----
            wq_sb = [pers_p.tile([P128, E], RD, name=f"wq{i}") for i in range(ET)]
            wp_sb = [pers_p.tile([P128, E], RD, name=f"wp{i}") for i in range(ET)]
            wp_last = pers_p.tile([1, E], RD)
            for i in range(ET):
                nc.sync.dma_start(wq_sb[i][:], wqT_d[i * P128:(i + 1) * P128, :])

            kT_sb = [pers_p.tile([P128, SP + P], RD, name=f"kT{i}") for i in range(ET)]
            v_sb = [pers_p.tile([P128, VA], RD, name=f"vsb{i}") for i in range(NT)]
            imp_sb = pers_p.tile([P128, T // P128], F32)

            # ================= Phase A: kv projection (sharded) ===========
            with ExitStack() as actx:
                akvc_p = actx.enter_context(tc.tile_pool(name="akvc", bufs=KCT + 1))
                aw_p = actx.enter_context(tc.tile_pool(name="aw", bufs=3))
                aps_p = actx.enter_context(
                    tc.tile_pool(name="aps", bufs=8, space="PSUM"))
                asb_p = actx.enter_context(tc.tile_pool(name="asb", bufs=4))

                kvc = []
                for kc in range(KCT + 1):
                    pr = P128 if kc < KCT else 1
                    t = akvc_p.tile([pr, SP + P], RD, name=f"kvc{kc}", tag="kvc")
                    nc.sync.dma_start(t[:], kvcT_d[kc * P128:kc * P128 + pr, :])
                    kvc.append(t)

                kT_in = dram_p.tile([E, SP + P], F32)
                v_in = dram_p.tile([SP, VA], F32)
                if use_cc:
                    kT_out = dram_p.tile([E, SP + P], F32)
                    v_out = dram_p.tile([SP, VA], F32)
                else:
                    kT_out, v_out = kT_in, v_in

                # A1: kT[o, n] = sum_i wkT[i, o] * kvcT[i, n]  (segment cols)
                kps = [aps_p.tile([P128, SP], F32, name=f"kps{o}", tag="ps")
                       for o in range(ET)]
                for kc in range(KCT + 1):
                    pr = P128 if kc < KCT else 1
                    wk = aw_p.tile([pr, E], RD, name="wk", tag="wk")
                    nc.sync.dma_start(wk[:], wkT_d[kc * P128:kc * P128 + pr, :])
                    for o in range(ET):
                        nc.tensor.matmul(
                            kps[o][:], wk[:, o * P128:(o + 1) * P128],
                            kvc[kc][:, :SP], start=(kc == 0), stop=(kc == KCT))
                # A1b: probe cols use the probe weight (K for batch0, V for batch1)
                pps = [aps_p.tile([P128, P], F32, name=f"pps{o}", tag="ps")
                       for o in range(ET)]
                for kc in range(KCT + 1):
                    pr = P128 if kc < KCT else 1
                    wpk = aw_p.tile([pr, E], RD, name="wpk", tag="wpk")
                    nc.sync.dma_start(wpk[:], wpkT_d[kc * P128:kc * P128 + pr, :])
                    for o in range(ET):
                        nc.tensor.matmul(
                            pps[o][:], wpk[:, o * P128:(o + 1) * P128],
                            kvc[kc][:, SP:SP + P], start=(kc == 0),
                            stop=(kc == KCT))
                for o in range(ET):
                    ksb = asb_p.tile([P128, SP + P], F32, name="ksb", tag="ksb")
                    nc.vector.tensor_copy(ksb[:, :SP], kps[o][:])
                    nc.vector.tensor_copy(ksb[:, SP:], pps[o][:])
                    nc.scalar.dma_start(kT_in[o * P128:(o + 1) * P128, :], ksb[:])

                # A2: v[n, j] = sum_i kvcT[i, n] * wvT[i, j]
                vps = [aps_p.tile([P128, VA // 2], F32, name=f"vps{i}", tag="ps")
                       for i in range(NT * 2)]
                for kc in range(KCT + 1):
                    pr = P128 if kc < KCT else 1
                    wv = aw_p.tile([pr, VA], RD, name="wv", tag="wv")
                    nc.sync.dma_start(wv[:], wvT_d[kc * P128:kc * P128 + pr, :])
                    for n in range(NT):
                        for hf in range(2):
                            nc.tensor.matmul(
                                vps[n * 2 + hf][:],
                                kvc[kc][:, n * P128:(n + 1) * P128],
                                wv[:, hf * (VA // 2):(hf + 1) * (VA // 2)],
                                start=(kc == 0), stop=(kc == KCT))
                for n in range(NT):
                    vsb = asb_p.tile([P128, VA], F32, name="vsb", tag="vsb")
                    for hf in range(2):
                        nc.vector.tensor_copy(
                            vsb[:, hf * (VA // 2):(hf + 1) * (VA // 2)],
                            vps[n * 2 + hf][:])
                    nc.scalar.dma_start(v_in[n * P128:(n + 1) * P128, :], vsb[:])

                if use_cc:
                    nc.gpsimd.collective_compute(
                        "AllReduce", mybir.AluOpType.add, replica_groups=groups,
                        ins=[kT_in[:]], outs=[kT_out[:]])
                    nc.gpsimd.collective_compute(
                        "AllReduce", mybir.AluOpType.add, replica_groups=groups,
                        ins=[v_in[:]], outs=[v_out[:]])

                for o in range(ET):
                    nc.sync.dma_start(
                        kT_sb[o][:], kT_out[o * P128:(o + 1) * P128, :].bitcast(RD))
                for n in range(NT):
                    nc.sync.dma_start(
                        v_sb[n][:], v_out[n * P128:(n + 1) * P128, :].bitcast(RD))

            for i in range(ET):
                nc.scalar.dma_start(wp_sb[i][:], wpT_d[i * P128:(i + 1) * P128, :])
            nc.scalar.dma_start(wp_last[:], wpT_d[E:E + 1, :])

            # ================= Phase B: main pipeline =====================
            qt_p = ctx.enter_context(tc.tile_pool(name="qt", bufs=2))
            qp_p = ctx.enter_context(tc.tile_pool(name="qp", bufs=B_qp))
            pt_p = ctx.enter_context(tc.tile_pool(name="pt", bufs=B_pt))
            ot_p = ctx.enter_context(tc.tile_pool(name="ot", bufs=B_ot))
            sm_p = ctx.enter_context(tc.tile_pool(name="sm", bufs=3))
            pr_p = ctx.enter_context(tc.tile_pool(name="pr", bufs=3))
            ysb_p = ctx.enter_context(tc.tile_pool(name="ysb", bufs=B_ysb))
            ps_p = ctx.enter_context(tc.tile_pool(name="ps", bufs=B_ps, space="PSUM"))
            op_p = ctx.enter_context(tc.tile_pool(name="op", bufs=B_op, space="PSUM"))

            qpTs = {}
            for tcn in range(TC):
                t0 = tcn * 512
                qT = [qt_p.tile([P128, 512], RD, name=f"qT{i}", tag=f"qT{i}")
                      for i in range(ET)]
                for i in range(ET):
                    nc.scalar.dma_start(qT[i][:],
                                        qT_d[i * P128:(i + 1) * P128, t0:t0 + 512])
                qpT = [qp_p.tile([P128, 512], RD, name=f"qpT{i}", tag=f"qpT{i}")
                       for i in range(ET)]
                qpTs[tcn] = qpT
                for o in range(ET):
                    qps = ps_p.tile([P128, 512], F32, name="qps", tag="ps")
                    for ic in range(ET):
                        nc.tensor.matmul(
                            qps[:], wq_sb[ic][:, o * P128:(o + 1) * P128],
                            qT[ic][:], start=(ic == 0), stop=(ic == ET - 1))
                    nc.scalar.activation(
                        qpT[o][:], qps[:], mybir.ActivationFunctionType.Identity,
                        bias=bq_sb[:, o:o + 1])

            for tcn in range(TC):
                t0 = tcn * 512
                qpT = qpTs[tcn]
                # ---- attention: logits+exp+denominators, then AV ----
                outT = [ot_p.tile([P128, 512], RD, name=f"outT{i}", tag=f"outT{i}")
                        for i in range(ET)]
                for h in range(H):
                    et, ro = (h * D) // P128, (h * D) % P128
                    qph = qpT[et][ro:ro + D, :]
                    PT = [pt_p.tile([P128, 512], RD, name=f"PT{n}",
                                    tag=f"PT{n}") for n in range(NT)]
                    for n in range(NT):
                        lt = ps_p.tile([P128, 512], F32, name="lt", tag="ps")
                        nc.tensor.matmul(
                            lt[:],
                            kT_sb[et][ro:ro + D, n * P128:(n + 1) * P128],
                            qph, start=True, stop=True)
                        nc.scalar.activation(
                            PT[n][:], lt[:],
                            mybir.ActivationFunctionType.Exp,
                            bias=mask_sb[:, n:n + 1], scale=float(scale))
                    op = op_p.tile([HD1, 512], F32, name="op", tag="op")
                    for n in range(NT):
                        nc.tensor.matmul(
                            op[:], v_sb[n][:, h * HD1:(h + 1) * HD1],
                            PT[n][:], start=(n == 0), stop=(n == NT - 1))
                    rden = sm_p.tile([1, 512], F32, name="rden", tag="rden")
                    nc.vector.reciprocal(rden[:], op[D:D + 1, :])
                    rbc = sm_p.tile([D, 512], F32, name="rbc", tag="rbc")
                    nc.gpsimd.partition_broadcast(rbc[:], rden[:])
                    nc.vector.tensor_mul(outT[et][ro:ro + D, :],
                                         op[0:D, :], rbc[:])

                # ---- probes: logits [t, 4, P] per head, batched stats ----
                maxPH = pr_p.tile([P128, 4, H], F32, name="maxPH", tag="maxPH")
                denPH = pr_p.tile([P128, 4, H], F32, name="denPH", tag="denPH")
                for h in range(H):
                    et, ro = (h * D) // P128, (h * D) % P128
                    pl = ps_p.tile([P128, 4, P], F32, name="pl", tag="ps")
                    for tt in range(4):
                        nc.tensor.matmul(
                            pl[:, tt, :],
                            qpT[et][ro:ro + D, tt * P128:(tt + 1) * P128],
                            kT_sb[et][ro:ro + D, SP:SP + P],
                            start=True, stop=True)
                    sp = pr_p.tile([P128, 4, P], F32, name="sp", tag="sp")
                    nc.scalar.activation(
                        sp[:], pl[:], mybir.ActivationFunctionType.Exp)
                    nc.vector.tensor_reduce(
                        maxPH[:, :, h], sp[:], axis=mybir.AxisListType.X,
                        op=mybir.AluOpType.max)
                    nc.vector.tensor_reduce(
                        denPH[:, :, h], sp[:], axis=mybir.AxisListType.X,
                        op=mybir.AluOpType.add)
                rcp = pr_p.tile([P128, 4, H], F32, name="rcp", tag="rcp")
                nc.vector.reciprocal(rcp[:], denPH[:])
                iv = pr_p.tile([P128, 4, H], F32, name="iv", tag="iv")
                nc.vector.tensor_mul(iv[:], maxPH[:], rcp[:])
                red = pr_p.tile([P128, 4], F32, name="red", tag="red")
                nc.vector.tensor_reduce(
                    red[:], iv[:], axis=mybir.AxisListType.X,
                    op=mybir.AluOpType.add)
                nc.vector.tensor_scalar_mul(
                    imp_sb[:, tcn * 4:tcn * 4 + 4], red[:], 1.0 / H)

                # ---- output projection ----
                for tt in range(4):
                    ysb = ysb_p.tile([P128, E], F32, name="ysb", tag="ysb")
                    for hf in range(2):
                        yps = ps_p.tile([P128, E // 2], F32, name="yps", tag="ps")
                        for ec in range(ET):
                            nc.tensor.matmul(
                                yps[:],
                                outT[ec][:, tt * P128:(tt + 1) * P128],
                                wp_sb[ec][:, hf * (E // 2):(hf + 1) * (E // 2)],
                                start=(ec == 0), stop=False)
                        nc.tensor.matmul(
                            yps[:], ones_row[:, tt * P128:(tt + 1) * P128],
                            wp_last[:, hf * (E // 2):(hf + 1) * (E // 2)],
                            start=False, stop=True)
                        nc.scalar.copy(ysb[:, hf * (E // 2):(hf + 1) * (E // 2)],
                                       yps[:])
                    nc.scalar.dma_start(
                        y_d[t0 + tt * P128:t0 + (tt + 1) * P128, :], ysb[:])

            nc.scalar.dma_start(imp_d.rearrange("a b -> b a"), imp_sb[:])

    nc.compile()
    return nc


def kernel(q, kv_compact, W_q, b_q, W_kv, v_bias, W_proj, b_proj,
           cu_seqlens_k, max_seqlen_k, probe_ids):
    q = np.asarray(q, np.float32)
    kv_compact = np.asarray(kv_compact, np.float32)
    W_q = np.asarray(W_q, np.float32)
    b_q = np.asarray(b_q, np.float32)
    W_kv = np.asarray(W_kv, np.float32)
    v_bias = np.asarray(v_bias, np.float32)
    W_proj = np.asarray(W_proj, np.float32)
    b_proj = np.asarray(b_proj, np.float32)
    cu = np.asarray(cu_seqlens_k, np.int64)
    probe_ids = np.asarray(probe_ids, np.int64)

    B, LQ, E = q.shape
    N, KVD = kv_compact.shape
    H = 12
    D = E // H
    assert B == 2 and NCORES % B == 0
    CPB = NCORES // B            # cores per batch
    T = (B * LQ) // NCORES       # tokens per core
    P = len(probe_ids)
    KC = KVD // CPB
    S = [int(cu[b + 1] - cu[b]) for b in range(B)]
    SP = max(P128, -(-max(S) // P128) * P128)

    # ---------- host-side input prep ----------
    kvT = np.ascontiguousarray(kv_compact.T)               # [KVD, N]
    probeT = kvT[:, probe_ids]                             # [KVD, P]
    wkT_full = np.ascontiguousarray(W_kv[:E].T)            # [KVD, E]
    wvT_full = np.ascontiguousarray(W_kv[E:].T)            # [KVD, E]
    wqT = np.ascontiguousarray(W_q.T)
    HD1 = D + 1
    VA = H * HD1
    wvT_aug = np.zeros((KVD + 1, VA), np.float32)
    for h in range(H):
        wvT_aug[:KVD, h * HD1:h * HD1 + D] = wvT_full[:, h * D:(h + 1) * D]
        wvT_aug[KVD, h * HD1 + D] = 1.0
    b_eff = W_proj @ v_bias + b_proj
    wpT = np.concatenate([np.ascontiguousarray(W_proj.T), b_eff[None, :]], 0)

    in_maps = []
    for c in range(NCORES):
        b, g = c // CPB, c % CPB
        lo, hi = int(cu[b]), int(cu[b + 1])
        segT = np.zeros((KVD, SP), np.float32)
        segT[:, :hi - lo] = kvT[:, lo:hi]
        kvcT_aug = np.concatenate([segT, probeT], 1)       # [KVD, SP+P]
        kvcT = np.zeros((KC + 1, SP + P), np.float32)
        kvcT[:KC] = kvcT_aug[g * KC:(g + 1) * KC]
        if g == 0:
            kvcT[KC] = 1.0    # ones row: v denominator + probe v_bias source
        wkT = np.zeros((KC + 1, E), np.float32)
        wkT[:KC] = wkT_full[g * KC:(g + 1) * KC]
        wvT = np.zeros((KC + 1, VA), np.float32)
        wvT[:KC] = wvT_aug[g * KC:(g + 1) * KC]
        if g == 0:
            wvT[KC] = wvT_aug[KVD]
        # probe projection: batch 0 probes against K, batch 1 against V(+bias)
        wpk_full = wkT_full if b == 0 else wvT_full
        wpkT = np.zeros((KC + 1, E), np.float32)
        wpkT[:KC] = wpk_full[g * KC:(g + 1) * KC]
        if g == 0 and b == 1:
            wpkT[KC] = v_bias
        mask = np.zeros((SP // P128, P128), np.float32)
        mask.reshape(-1)[hi - lo:] = -1e30
        in_maps.append({
            "qT_s": np.ascontiguousarray(q[b, g * T:(g + 1) * T].T),
            "kvcT": kvcT, "wkT": wkT, "wvT": wvT, "wpkT": wpkT,
            "wqT": wqT, "wpT": wpT, "maskns": mask,
            "bq": np.ascontiguousarray(b_q.reshape(E // P128, P128)),
            "ones": np.ones((1, 512), np.float32),
        })

    nc = _build(T, SP, P, KC, E, H, D, True)
    res = run_bass_kernel_spmd(nc, in_maps, core_ids=list(range(NCORES)))

    out = np.empty((B, LQ, E), np.float32)
    imp = np.empty((B, LQ), np.float32)
    for c in range(NCORES):
        b, g = c // CPB, c % CPB
        out[b, g * T:(g + 1) * T] = res.results[c]["y_s"]
        imp[b, g * T:(g + 1) * T] = res.results[c]["imp_s"].reshape(-1)
    return out, imp


# revision 22
# speedup vs baseline: 1.1591x; 1.0102x over previous
"""CrossAttnBlock Trainium2 kernel (8 NeuronCores, SPMD).

Sharding: query-sequence parallel. 16384 query rows (B*Lq) are split 8 ways;
cores 0-3 take batch 0, cores 4-7 take batch 1 (2048 rows each). The small
ragged KV (N=512) projection is contraction-sharded 4 ways inside each batch
group and AllReduce'd; each core only materializes its own batch's KV segment
plus the probe columns.

Layout plan (per core, T=2048 query tokens):
  qT   = host-pretransposed q shard               [E, T]    (f32r, direct DMA)
  qpT  = W_qT.T @ qT + b_q (per-partition add)    [E, T]    psum->sbuf
  kT   = kv-proj, k half, transposed layout       [E, S+P]  (AllReduce'd)
         probe cols use W_k (batch 0) / W_v + v_bias (batch 1) -- the
         reference's einsum 'bhqd,bhpd' binds batch to the k/v axis of kv.
  v    = kv-proj, v half, natural layout          [S, E]    (AllReduce'd)
  LT   = kT_h.T @ qpT_h per head                  [S, T]    psum (transposed)
  PT   = exp(scale*LT + mask_bias)                [S, T]    f32r sbuf
  den  = ones.T @ PT per head -> one [H, T] psum; single reciprocal
  oT   = v_h.T @ PT (unnormalized)                [D, T]    psum
  out  = oT * gpsimd-broadcast(1/den_h)           [E, T]    sbuf
  y    = outT.T @ W_projT (+b_eff via ones row)   [T, E]    psum->sbuf->DRAM
  probes: natural-layout logits [T, 4, P] per head, one exp per head-chunk,
          batched free-dim max/sum reduces -> imp = mean_h(max/den).

v_bias enters the attention output only through b_eff = W_proj@v_bias + b_proj
(softmax rows sum to 1), applied via an appended ones row on the proj
stationary. All PE-facing tensors are float32r (fp32 bits, reduced-precision
full-rate streaming); PSUM accumulation stays fp32.
"""

import functools
import sys

import numpy as np

try:
    import concourse.bass as bass  # noqa: F401
except ImportError:
    sys.path.insert(0, "/opt/trn_rl_repo")

import concourse.bass as bass
import concourse.tile as tile
from concourse import bacc, mybir
from concourse.bass_utils import run_bass_kernel_spmd

F32 = mybir.dt.float32
F32R = mybir.dt.float32r

NCORES = 8
P128 = 128


@functools.lru_cache(maxsize=4)
def _build(T, SP, P, KC, E, H, D, use_f32r, use_cc=True, bufs_cfg=()):
    """Build + compile the per-core Bass module. All shapes are compile-time.

    T  : query tokens per core (2048)
    SP : padded KV segment length (mult of 128)
    P  : number of probe ids
    KC : contraction rows of the kv projection handled per core (KVD/4)
    """
    assert T % 512 == 0 and SP % P128 == 0 and E % P128 == 0
    HD1 = D + 1         # per-head v width incl. ones column
    VA = H * HD1        # 780
    NT = SP // P128     # kv n-tiles (2)
    ET = E // P128      # feature tiles (6)
    TC = T // 512       # 512-token chunks (4)
    KCT = KC // P128    # kv-proj K tiles per core (8) + 1 single-row
    scale = 1.0 / np.sqrt(D)
    RD = F32R if use_f32r else F32
    cfg = dict(bufs_cfg)
    B_qp = cfg.get("qp", 5); B_pt = cfg.get("pt", 3); B_ot = cfg.get("ot", 2)
    B_ps = cfg.get("ps", 6); B_op = cfg.get("op", 2); B_ysb = cfg.get("ysb", 3)
    HB = cfg.get("HB", 4)

    nc = bacc.Bacc("TRN2", target_bir_lowering=False, debug=False,
                   num_devices=NCORES)

    qT_d = nc.dram_tensor("qT_s", [E, T], RD, kind="ExternalInput").ap()
    kvcT_d = nc.dram_tensor("kvcT", [KC + 1, SP + P], RD, kind="ExternalInput").ap()
    wkT_d = nc.dram_tensor("wkT", [KC + 1, E], RD, kind="ExternalInput").ap()
    wvT_d = nc.dram_tensor("wvT", [KC + 1, VA], RD, kind="ExternalInput").ap()
    wpkT_d = nc.dram_tensor("wpkT", [KC + 1, E], RD, kind="ExternalInput").ap()
    wqT_d = nc.dram_tensor("wqT", [E, E], RD, kind="ExternalInput").ap()
    wpT_d = nc.dram_tensor("wpT", [E + 1, E], RD, kind="ExternalInput").ap()
    bq_d = nc.dram_tensor("bq", [E // P128, P128], F32, kind="ExternalInput").ap()
    mask_d = nc.dram_tensor("maskns", [SP // P128, P128], F32,
                            kind="ExternalInput").ap()
    ones_d = nc.dram_tensor("ones", [1, 512], RD, kind="ExternalInput").ap()
    y_d = nc.dram_tensor("y_s", [T, E], F32, kind="ExternalOutput").ap()
    imp_d = nc.dram_tensor("imp_s", [T // P128, P128], F32, kind="ExternalOutput").ap()

    groups = [[0, 1, 2, 3], [4, 5, 6, 7]]

    with tile.TileContext(nc) as tc:
        from contextlib import ExitStack
        with ExitStack() as ctx:
            const_p = ctx.enter_context(tc.tile_pool(name="const", bufs=1))
            pers_p = ctx.enter_context(tc.tile_pool(name="pers", bufs=1))
            dram_p = ctx.enter_context(tc.tile_pool(name="dram", bufs=1, space="DRAM"))

            ones_row = const_p.tile([1, 512], RD)
            nc.sync.dma_start(ones_row[:], ones_d[:])
            ones_col = const_p.tile([P128, 1], RD)
            nc.sync.dma_start(ones_col[:], ones_d[0, :P128])
            mask_sb = const_p.tile([P128, NT], F32)
            for j in range(NT):
                nc.sync.dma_start(mask_sb[:, j:j + 1], mask_d[j, :])
            bq_sb = const_p.tile([P128, ET], F32)
            for j in range(ET):
                nc.sync.dma_start(bq_sb[:, j:j + 1], bq_d[j, :])

            # ---- persistent tensors ----
            wq_sb = [pers_p.tile([P128, E], RD, name=f"wq{i}") for i in range(ET)]
            wp_sb = [pers_p.tile([P128, E], RD, name=f"wp{i}") for i in range(ET)]
            wp_last = pers_p.tile([1, E], RD)

            kT_sb = [pers_p.tile([P128, SP + P], RD, name=f"kT{i}") for i in range(ET)]
            v_sb = [pers_p.tile([P128, VA], RD, name=f"vsb{i}") for i in range(NT)]
            imp_sb = pers_p.tile([P128, T // P128], F32)

            # ================= Phase A: kv projection (sharded) ===========
            with ExitStack() as actx:
                akvc_p = actx.enter_context(tc.tile_pool(name="akvc", bufs=KCT + 1))
                aw_p = actx.enter_context(tc.tile_pool(name="aw", bufs=3))
                aps_p = actx.enter_context(
                    tc.tile_pool(name="aps", bufs=8, space="PSUM"))
                asb_p = actx.enter_context(tc.tile_pool(name="asb", bufs=4))

                kvc = []
                for kc in range(KCT + 1):
                    pr = P128 if kc < KCT else 1
                    t = akvc_p.tile([pr, SP + P], RD, name=f"kvc{kc}", tag="kvc")
                    nc.sync.dma_start(t[:], kvcT_d[kc * P128:kc * P128 + pr, :])
                    kvc.append(t)

                kT_in = dram_p.tile([E, SP + P], F32)
                v_in = dram_p.tile([SP, VA], F32)
                if use_cc:
                    kT_out = dram_p.tile([E, SP + P], F32)
                    v_out = dram_p.tile([SP, VA], F32)
                else:
                    kT_out, v_out = kT_in, v_in

                for i in range(ET):
                    nc.scalar.dma_start(wq_sb[i][:],
                                        wqT_d[i * P128:(i + 1) * P128, :])

                # A1: kT[o, n] = sum_i wkT[i, o] * kvcT[i, n]  (segment cols)
                kps = [aps_p.tile([P128, SP], F32, name=f"kps{o}", tag="ps")
                       for o in range(ET)]
                for kc in range(KCT + 1):
                    pr = P128 if kc < KCT else 1
                    wk = aw_p.tile([pr, E], RD, name="wk", tag="wk")
                    nc.sync.dma_start(wk[:], wkT_d[kc * P128:kc * P128 + pr, :])
                    for o in range(ET):
                        nc.tensor.matmul(
                            kps[o][:], wk[:, o * P128:(o + 1) * P128],
                            kvc[kc][:, :SP], start=(kc == 0), stop=(kc == KCT))
                # A1b: probe cols use the probe weight (K for batch0, V for batch1)
                pps = [aps_p.tile([P128, P], F32, name=f"pps{o}", tag="ps")
                       for o in range(ET)]
                for kc in range(KCT + 1):
                    pr = P128 if kc < KCT else 1
                    wpk = aw_p.tile([pr, E], RD, name="wpk", tag="wpk")
                    nc.sync.dma_start(wpk[:], wpkT_d[kc * P128:kc * P128 + pr, :])
                    for o in range(ET):
                        nc.tensor.matmul(
                            pps[o][:], wpk[:, o * P128:(o + 1) * P128],
                            kvc[kc][:, SP:SP + P], start=(kc == 0),
                            stop=(kc == KCT))
                for o in range(ET):
                    ksb = asb_p.tile([P128, SP + P], F32, name="ksb", tag="ksb")
                    nc.vector.tensor_copy(ksb[:, :SP], kps[o][:])
                    nc.vector.tensor_copy(ksb[:, SP:], pps[o][:])
                    nc.scalar.dma_start(kT_in[o * P128:(o + 1) * P128, :], ksb[:])

                # A2: v[n, j] = sum_i kvcT[i, n] * wvT[i, j]
                vps = [aps_p.tile([P128, VA // 2], F32, name=f"vps{i}", tag="ps")
                       for i in range(NT * 2)]
                for kc in range(KCT + 1):
                    pr = P128 if kc < KCT else 1
                    wv = aw_p.tile([pr, VA], RD, name="wv", tag="wv")
                    nc.sync.dma_start(wv[:], wvT_d[kc * P128:kc * P128 + pr, :])
                    for n in range(NT):
                        for hf in range(2):
                            nc.tensor.matmul(
                                vps[n * 2 + hf][:],
                                kvc[kc][:, n * P128:(n + 1) * P128],
                                wv[:, hf * (VA // 2):(hf + 1) * (VA // 2)],
                                start=(kc == 0), stop=(kc == KCT))
                for n in range(NT):
                    vsb = asb_p.tile([P128, VA], F32, name="vsb", tag="vsb")
                    for hf in range(2):
                        nc.vector.tensor_copy(
                            vsb[:, hf * (VA // 2):(hf + 1) * (VA // 2)],
                            vps[n * 2 + hf][:])
                    nc.scalar.dma_start(v_in[n * P128:(n + 1) * P128, :], vsb[:])

                if use_cc:
                    nc.gpsimd.collective_compute(
                        "AllReduce", mybir.AluOpType.add, replica_groups=groups,
                        ins=[kT_in[:]], outs=[kT_out[:]])
                    nc.gpsimd.collective_compute(
                        "AllReduce", mybir.AluOpType.add, replica_groups=groups,
                        ins=[v_in[:]], outs=[v_out[:]])

                for o in range(ET):
                    nc.sync.dma_start(
                        kT_sb[o][:], kT_out[o * P128:(o + 1) * P128, :].bitcast(RD))
                for n in range(NT):
                    nc.sync.dma_start(
                        v_sb[n][:], v_out[n * P128:(n + 1) * P128, :].bitcast(RD))

            for i in range(ET):
                nc.scalar.dma_start(wp_sb[i][:], wpT_d[i * P128:(i + 1) * P128, :])
            nc.scalar.dma_start(wp_last[:], wpT_d[E:E + 1, :])

            # ================= Phase B: main pipeline =====================
            qt_p = ctx.enter_context(tc.tile_pool(name="qt", bufs=2))
            qp_p = ctx.enter_context(tc.tile_pool(name="qp", bufs=B_qp))
            pt_p = ctx.enter_context(tc.tile_pool(name="pt", bufs=B_pt))
            ot_p = ctx.enter_context(tc.tile_pool(name="ot", bufs=B_ot))
            sm_p = ctx.enter_context(tc.tile_pool(name="sm", bufs=3))
            pr_p = ctx.enter_context(tc.tile_pool(name="pr", bufs=3))
            ysb_p = ctx.enter_context(tc.tile_pool(name="ysb", bufs=B_ysb))
            ps_p = ctx.enter_context(tc.tile_pool(name="ps", bufs=B_ps, space="PSUM"))
            op_p = ctx.enter_context(tc.tile_pool(name="op", bufs=B_op, space="PSUM"))

            qpTs = {}
            for tcn in range(TC):
                t0 = tcn * 512
                qT = [qt_p.tile([P128, 512], RD, name=f"qT{i}", tag=f"qT{i}")
                      for i in range(ET)]
                for i in range(ET):
                    nc.scalar.dma_start(qT[i][:],
                                        qT_d[i * P128:(i + 1) * P128, t0:t0 + 512])
                qpT = [qp_p.tile([P128, 512], RD, name=f"qpT{i}", tag=f"qpT{i}")
                       for i in range(ET)]
                qpTs[tcn] = qpT
                for o in range(ET):
                    qps = ps_p.tile([P128, 512], F32, name="qps", tag="ps")
                    for ic in range(ET):
                        nc.tensor.matmul(
                            qps[:], wq_sb[ic][:, o * P128:(o + 1) * P128],
                            qT[ic][:], start=(ic == 0), stop=(ic == ET - 1))
                    nc.scalar.activation(
                        qpT[o][:], qps[:], mybir.ActivationFunctionType.Identity,
                        bias=bq_sb[:, o:o + 1])

            for tcn in range(TC):
                t0 = tcn * 512
                qpT = qpTs[tcn]
                # ---- attention: logits+exp+denominators, then AV ----
                outT = [ot_p.tile([P128, 512], RD, name=f"outT{i}", tag=f"outT{i}")
                        for i in range(ET)]
                for h in range(H):
                    et, ro = (h * D) // P128, (h * D) % P128
                    qph = qpT[et][ro:ro + D, :]
                    PT = [pt_p.tile([P128, 512], RD, name=f"PT{n}",
                                    tag=f"PT{n}") for n in range(NT)]
                    for n in range(NT):
                        lt = ps_p.tile([P128, 512], F32, name="lt", tag="ps")
                        nc.tensor.matmul(
                            lt[:],
                            kT_sb[et][ro:ro + D, n * P128:(n + 1) * P128],
                            qph, start=True, stop=True)
                        nc.scalar.activation(
                            PT[n][:], lt[:],
                            mybir.ActivationFunctionType.Exp,
                            bias=mask_sb[:, n:n + 1], scale=float(scale))
                    op = op_p.tile([HD1, 512], F32, name="op", tag="op")
                    for n in range(NT):
                        nc.tensor.matmul(
                            op[:], v_sb[n][:, h * HD1:(h + 1) * HD1],
                            PT[n][:], start=(n == 0), stop=(n == NT - 1))
                    rden = sm_p.tile([1, 512], F32, name="rden", tag="rden")
                    nc.vector.reciprocal(rden[:], op[D:D + 1, :])
                    rbc = sm_p.tile([D, 512], F32, name="rbc", tag="rbc")
                    nc.gpsimd.partition_broadcast(rbc[:], rden[:])
                    nc.vector.tensor_mul(outT[et][ro:ro + D, :],
                                         op[0:D, :], rbc[:])

                # ---- probes: logits [t, 4, P] per head, batched stats ----
                maxPH = pr_p.tile([P128, 4, H], F32, name="maxPH", tag="maxPH")
                denPH = pr_p.tile([P128, 4, H], F32, name="denPH", tag="denPH")
                for h in range(H):
                    et, ro = (h * D) // P128, (h * D) % P128
                    pl = ps_p.tile([P128, 4, P], F32, name="pl", tag="ps")
                    for tt in range(4):
                        nc.tensor.matmul(
                            pl[:, tt, :],
                            qpT[et][ro:ro + D, tt * P128:(tt + 1) * P128],
                            kT_sb[et][ro:ro + D, SP:SP + P],
                            start=True, stop=True)
                    sp = pr_p.tile([P128, 4, P], F32, name="sp", tag="sp")
                    nc.scalar.activation(
                        sp[:], pl[:], mybir.ActivationFunctionType.Exp)
                    nc.vector.tensor_reduce(
                        maxPH[:, :, h], sp[:], axis=mybir.AxisListType.X,
                        op=mybir.AluOpType.max)
                    nc.vector.tensor_reduce(
                        denPH[:, :, h], sp[:], axis=mybir.AxisListType.X,
                        op=mybir.AluOpType.add)
                rcp = pr_p.tile([P128, 4, H], F32, name="rcp", tag="rcp")
                nc.vector.reciprocal(rcp[:], denPH[:])
                iv = pr_p.tile([P128, 4, H], F32, name="iv", tag="iv")
                nc.vector.tensor_mul(iv[:], maxPH[:], rcp[:])
                red = pr_p.tile([P128, 4], F32, name="red", tag="red")
                nc.vector.tensor_reduce(
                    red[:], iv[:], axis=mybir.AxisListType.X,
                    op=mybir.AluOpType.add)
                nc.vector.tensor_scalar_mul(
                    imp_sb[:, tcn * 4:tcn * 4 + 4], red[:], 1.0 / H)

                # ---- output projection ----
                for tt in range(4):
                    ysb = ysb_p.tile([P128, E], F32, name="ysb", tag="ysb")
                    for hf in range(2):
                        yps = ps_p.tile([P128, E // 2], F32, name="yps", tag="ps")
                        for ec in range(ET):
                            nc.tensor.matmul(
                                yps[:],
                                outT[ec][:, tt * P128:(tt + 1) * P128],
                                wp_sb[ec][:, hf * (E // 2):(hf + 1) * (E // 2)],
                                start=(ec == 0), stop=False)
                        nc.tensor.matmul(
                            yps[:], ones_row[:, tt * P128:(tt + 1) * P128],
                            wp_last[:, hf * (E // 2):(hf + 1) * (E // 2)],
                            start=False, stop=True)
                        nc.scalar.copy(ysb[:, hf * (E // 2):(hf + 1) * (E // 2)],
                                       yps[:])
                    nc.scalar.dma_start(
                        y_d[t0 + tt * P128:t0 + (tt + 1) * P128, :], ysb[:])

            nc.scalar.dma_start(imp_d.rearrange("a b -> b a"), imp_sb[:])

    nc.compile()
    return nc


def kernel(q, kv_compact, W_q, b_q, W_kv, v_bias, W_proj, b_proj,
           cu_seqlens_k, max_seqlen_k, probe_ids):
    q = np.asarray(q, np.float32)
    kv_compact = np.asarray(kv_compact, np.float32)
    W_q = np.asarray(W_q, np.float32)
    b_q = np.asarray(b_q, np.float32)
    W_kv = np.asarray(W_kv, np.float32)
    v_bias = np.asarray(v_bias, np.float32)
    W_proj = np.asarray(W_proj, np.float32)
    b_proj = np.asarray(b_proj, np.float32)
    cu = np.asarray(cu_seqlens_k, np.int64)
    probe_ids = np.asarray(probe_ids, np.int64)

    B, LQ, E = q.shape
    N, KVD = kv_compact.shape
    H = 12
    D = E // H
    assert B == 2 and NCORES % B == 0
    CPB = NCORES // B            # cores per batch
    T = (B * LQ) // NCORES       # tokens per core
    P = len(probe_ids)
    KC = KVD // CPB
    S = [int(cu[b + 1] - cu[b]) for b in range(B)]
    SP = max(P128, -(-max(S) // P128) * P128)

    # ---------- host-side input prep ----------
    kvT = np.ascontiguousarray(kv_compact.T)               # [KVD, N]
    probeT = kvT[:, probe_ids]                             # [KVD, P]
    wkT_full = np.ascontiguousarray(W_kv[:E].T)            # [KVD, E]
    wvT_full = np.ascontiguousarray(W_kv[E:].T)            # [KVD, E]
    wqT = np.ascontiguousarray(W_q.T)
    HD1 = D + 1
    VA = H * HD1
    wvT_aug = np.zeros((KVD + 1, VA), np.float32)
    for h in range(H):
        wvT_aug[:KVD, h * HD1:h * HD1 + D] = wvT_full[:, h * D:(h + 1) * D]
        wvT_aug[KVD, h * HD1 + D] = 1.0
    b_eff = W_proj @ v_bias + b_proj
    wpT = np.concatenate([np.ascontiguousarray(W_proj.T), b_eff[None, :]], 0)

    in_maps = []
    for c in range(NCORES):
        b, g = c // CPB, c % CPB
        lo, hi = int(cu[b]), int(cu[b + 1])
        segT = np.zeros((KVD, SP), np.float32)
        segT[:, :hi - lo] = kvT[:, lo:hi]
        kvcT_aug = np.concatenate([segT, probeT], 1)       # [KVD, SP+P]
        kvcT = np.zeros((KC + 1, SP + P), np.float32)
        kvcT[:KC] = kvcT_aug[g * KC:(g + 1) * KC]
        if g == 0:
            kvcT[KC] = 1.0    # ones row: v denominator + probe v_bias source
        wkT = np.zeros((KC + 1, E), np.float32)
        wkT[:KC] = wkT_full[g * KC:(g + 1) * KC]
        wvT = np.zeros((KC + 1, VA), np.float32)
        wvT[:KC] = wvT_aug[g * KC:(g + 1) * KC]
        if g == 0:
            wvT[KC] = wvT_aug[KVD]
        # probe projection: batch 0 probes against K, batch 1 against V(+bias)
        wpk_full = wkT_full if b == 0 else wvT_full
        wpkT = np.zeros((KC + 1, E), np.float32)
        wpkT[:KC] = wpk_full[g * KC:(g + 1) * KC]
        if g == 0 and b == 1:
            wpkT[KC] = v_bias
        mask = np.zeros((SP // P128, P128), np.float32)
        mask.reshape(-1)[hi - lo:] = -1e30
        in_maps.append({
            "qT_s": np.ascontiguousarray(q[b, g * T:(g + 1) * T].T),
            "kvcT": kvcT, "wkT": wkT, "wvT": wvT, "wpkT": wpkT,
            "wqT": wqT, "wpT": wpT, "maskns": mask,
            "bq": np.ascontiguousarray(b_q.reshape(E // P128, P128)),
            "ones": np.ones((1, 512), np.float32),
        })

    nc = _build(T, SP, P, KC, E, H, D, True)
    res = run_bass_kernel_spmd(nc, in_maps, core_ids=list(range(NCORES)))

    out = np.empty((B, LQ, E), np.float32)
    imp = np.empty((B, LQ), np.float32)
    for c in range(NCORES):
        b, g = c // CPB, c % CPB
        out[b, g * T:(g + 1) * T] = res.results[c]["y_s"]
        imp[b, g * T:(g + 1) * T] = res.results[c]["imp_s"].reshape(-1)
    return out, imp
